# revision 1
# baseline (speedup 1.0000x reference)
"""Trainium2 Bass kernel for nn_AttentionGuidedIterativeBlock.

Math reformulation: the (B,L,P,D) phasor cumsum + retrieval is causal linear
attention with feature map Kf = [cos(phases), sin(phases)] (2P=64 dims):

    retrieved[l] = (sum_{l'<=l} (Qf[l].Kf[l']) * V[l']) / (sqrt(l+1)*sqrt(P))

The K/V state is built once from x (it does not change across the I=3
refinement iterations); only Qf changes.  Sharding: 8 cores x 512 tokens
(cores 0-3 batch 0, 4-7 batch 1).  Each core rebuilds the prefix state
S = Kf_masked^T @ V over its batch (kmask zeroes tokens >= its segment),
then runs the 3 refinement iterations on its own 512 tokens, split into two
256-token halves that software-pipeline against each other (the refinement
is per-token independent; the K/V memory state is fixed).

LN gains/biases are folded into the following matmul weights on the host;
out_b is pre-added into the residual input on the host.
"""

import math
import os

import numpy as np

D, P, I, H = 256, 32, 3, 8
B, L = 2, 2048
NCORES = 8
SEG = 512          # tokens per core
HSEG = 256         # half-segment (pipelined unit)
CH = 128           # chunk (tile partition) size
NCH_B = L // CH    # 16 chunks per batch
NCH_S = SEG // CH  # 4 own chunks
PI = math.pi
EPS = 1e-5

_CACHE = {}


def _patch_walrus_passes():
    # float32r operands are fed raw fp32 bits (measured max rel err 4.2e-4
    # per matmul on HW); drop birverifier which insists producers round.
    import concourse.bass_utils as bu
    if getattr(bu, "_nv_patched", False):
        return
    orig = bu.run_command

    def patched(cmd, cwd=None, **kw):
        cmd = list(cmd)
        if "--pass" in cmd:
            i = cmd.index("--pass")
            cmd[i + 1] = cmd[i + 1].replace("birverifier,", "")
        return orig(cmd, cwd=cwd, **kw)

    bu.run_command = patched
    bu._nv_patched = True


def _build_program(split=True):
    _patch_walrus_passes()
    import concourse.bass as bass
    import concourse.tile as tile
    from concourse import mybir

    AF = mybir.ActivationFunctionType
    f32 = mybir.dt.float32
    f32r = mybir.dt.float32r

    if os.environ.get("MM_DTYPE", "f32r") == "f32":
        def r(ap):
            return ap
    else:
        def r(ap):  # bitcast fp32 AP to float32r for full-rate PE
            return ap.bitcast(f32r)

    nc = bass.Bass("TRN2", target_bir_lowering=False, debug=False,
                   num_devices=NCORES)

    def din(name, shape):
        return nc.dram_tensor(name, shape, f32, kind="ExternalInput").ap()

    t = {}
    t["x_pref_fm"] = din("x_pref_fm", (D, L))
    t["kmask"] = din("kmask", (L, 1))
    t["x_own_fm"] = din("x_own_fm", (D, SEG))
    t["x_own_tm"] = din("x_own_tm", (SEG, D))
    t["inv_norm"] = din("inv_norm", (2 * P, SEG))
    t["pe_w"] = din("pe_w", (D, P))
    t["pe_b_row"] = din("pe_b_row", (1, P))
    t["pe_b_col"] = din("pe_b_col", (P, 1))
    t["tv_w"] = din("tv_w", (D, D))
    t["tv_b_row"] = din("tv_b_row", (1, D))
    t["tvpe_w"] = din("tvpe_w", (D, D + P))
    t["tvpe_b"] = din("tvpe_b", (1, D + P))
    t["mq_w"] = din("mq_w", (D, H))
    t["mq_b_row"] = din("mq_b_row", (1, H))
    t["w1g"] = din("w1g", (I, D + H, 2 * D))
    t["b1e_t"] = din("b1e_t", (I, CH, 4))
    t["w2"] = din("w2", (I, 2 * D, D))
    t["b2_t"] = din("b2_t", (I, CH, 2))
    t["gate_w"] = din("gate_w", (I, 2 * D, D))
    t["gb_t"] = din("gb_t", (I, CH, 2))
    t["wog"] = din("wog", (D, D))
    t["ident"] = din("ident", (CH, CH))
    t["tril"] = din("tril", (CH, CH))
    t["y"] = nc.dram_tensor("y", (SEG, D), f32, kind="ExternalOutput").ap()

    with tile.TileContext(nc) as tc:
        _body(tc, nc, t, AF, f32, r, bass, mybir)
    if split:
        _split_waits(nc, mybir)
    return nc


def _split_waits(nc, mybir, cap=1):
    """This walrus build allows only one sync-wait slot per instruction
    (matmult lowers to LDW+MM where the LW struct carries the waits); move
    excess waits onto preceding same-engine NOPs."""
    for fn in nc.m.functions:
        for blk in fn.blocks:
            out = []
            for ins in blk.instructions:
                si = ins.sync_info
                if si is not None and len(si.on_wait) > cap:
                    waits = list(si.on_wait)
                    extra, keep = waits[:-cap], waits[-cap:]
                    for j, w in enumerate(extra):
                        nop = mybir.InstNoOp(name=f"{ins.name}_wsplit{j}",
                                             ins=[], outs=[])
                        nop.engine = ins.engine
                        nop.sync_info = mybir.SyncInfo(on_wait=[w],
                                                       on_update=[])
                        out.append(nop)
                    ins.sync_info = mybir.SyncInfo(on_wait=keep,
                                                   on_update=si.on_update)
                out.append(ins)
            blk.instructions = out


def _body(tc, nc, t, AF, f32, r, bass, mybir):
    from concourse.alu_op_type import AluOpType as OP

    AX = mybir.AxisListType.X

    consts = tc.alloc_tile_pool(name="consts", bufs=1)
    own = tc.alloc_tile_pool(name="own", bufs=1)
    pa = tc.alloc_tile_pool(name="pa", bufs=3)
    pb = tc.alloc_tile_pool(name="pb", bufs=3)
    psA = tc.alloc_tile_pool(name="psA", bufs=1, space="PSUM")

    dma = nc.sync.dma_start
    mm = nc.tensor.matmul

    # ---- constants / params in SBUF ----
    pe_w_sb = consts.tile([CH, 2, P], f32)
    dma(out=pe_w_sb, in_=t["pe_w"].rearrange("(c p) m -> p c m", c=2))
    tv_w_sb = consts.tile([CH, 2, D], f32)
    dma(out=tv_w_sb, in_=t["tv_w"].rearrange("(c p) m -> p c m", c=2))
    tvpe_sb = consts.tile([CH, 2, D + P], f32)
    dma(out=tvpe_sb, in_=t["tvpe_w"].rearrange("(c p) m -> p c m", c=2))
    tvpe_b_sb = consts.tile([1, D + P], f32)
    dma(out=tvpe_b_sb, in_=t["tvpe_b"])
    mq_w_sb = consts.tile([CH, 2, H], f32)
    dma(out=mq_w_sb, in_=t["mq_w"].rearrange("(c p) m -> p c m", c=2))
    wog_sb = consts.tile([CH, 2, D], f32)
    dma(out=wog_sb, in_=t["wog"].rearrange("(c p) m -> p c m", c=2))
    pe_b_row_sb = consts.tile([1, P], f32)
    dma(out=pe_b_row_sb, in_=t["pe_b_row"])
    pe_b_col_sb = consts.tile([P, 1], f32)
    dma(out=pe_b_col_sb, in_=t["pe_b_col"])
    tv_b_row_sb = consts.tile([1, D], f32)
    dma(out=tv_b_row_sb, in_=t["tv_b_row"])
    tv_b_bc64 = consts.tile([2 * P, D], f32)
    dma(out=tv_b_bc64, in_=t["tv_b_row"].to_broadcast((2 * P, D)))
    mq_b_bc = consts.tile([CH, H], f32)
    dma(out=mq_b_bc, in_=t["mq_b_row"].to_broadcast((CH, H)))
    ident_sb = consts.tile([CH, CH], f32)
    dma(out=ident_sb, in_=t["ident"])
    tril_sb = consts.tile([CH, CH], f32)
    dma(out=tril_sb, in_=t["tril"])
    inv_norm_sb = consts.tile([2 * P, SEG], f32)
    dma(out=inv_norm_sb, in_=t["inv_norm"])
    x_tm_sb = consts.tile([CH, NCH_S, D], f32)
    dma(out=x_tm_sb, in_=t["x_own_tm"].rearrange("(c p) m -> p c m", c=NCH_S))

    ones_row = consts.tile([1, CH], f32)
    nc.vector.memset(ones_row, 1.0)
    oc264 = consts.tile([CH, 1], f32)
    nc.vector.memset(oc264, 1.0 / (D + H))
    oc256 = consts.tile([CH, 1], f32)
    nc.vector.memset(oc256, 1.0 / D)
    halfpi = consts.tile([CH, 1], f32)
    nc.vector.memset(halfpi, PI / 2)
    epsb = consts.tile([CH, 1], f32)
    nc.vector.memset(epsb, EPS)

    # ---- own-segment K/V prep ----
    qA = own.tile([CH, 2, SEG], f32)
    dma(out=qA, in_=t["x_own_fm"].rearrange("(c p) l -> p c l", c=2))
    qB = own.tile([CH, 2, SEG], f32)

    qpo_ps = psA.tile([P, SEG], f32, tag="qpf")
    mm(qpo_ps, r(pe_w_sb[:, 0, :]), r(qA[:, 0, :]), start=True, stop=False)
    mm(qpo_ps, r(pe_w_sb[:, 1, :]), r(qA[:, 1, :]), start=False, stop=True)
    tqo = pb.tile([P, SEG], f32, tag="tq")
    nc.scalar.activation(tqo, qpo_ps, AF.Tanh, bias=pe_b_col_sb)
    aqo = pb.tile([P, SEG], f32, tag="aq")
    nc.scalar.activation(aqo, tqo, AF.Abs)
    kff = own.tile([2 * P, SEG], f32)
    nc.scalar.activation(kff[0:P, :], aqo, AF.Sin, scale=-PI,
                         bias=halfpi[0:P, :])
    nc.scalar.activation(kff[P:2 * P, :], tqo, AF.Sin, scale=PI)

    vo = own.tile([CH, NCH_S, D], f32)
    for c in range(NCH_S):
        vo_ps = psA.tile([CH, D], f32, tag="v_a", bufs=1, name=f"vo_ps{c}")
        sl = slice(c * CH, (c + 1) * CH)
        mm(vo_ps, r(qA[:, 0, sl]), r(tv_w_sb[:, 0, :]), start=True, stop=False)
        mm(vo_ps, r(qA[:, 1, sl]), r(tv_w_sb[:, 1, :]), start=False,
           stop=False)
        mm(vo_ps, r(ones_row), r(tv_b_row_sb), start=False, stop=True)
        nc.scalar.copy(vo[:, c, :], vo_ps)

    # ---- phase A: prefix state S = Kf_masked^T @ V_aug over the batch ----
    S_ps = psA.tile([2 * P, D + 8], f32, tag="S")
    for ci in range(NCH_B):
        xf = pa.tile([CH, 2, CH], f32, tag="xf")
        dma(out=xf, in_=t["x_pref_fm"].rearrange("(c p) l -> p c l", c=2)
            [:, :, ci * CH:(ci + 1) * CH])
        vq_ps = psA.tile([CH, D + P], f32, tag="qp_a", bufs=2, name="vq_ps")
        mm(vq_ps, r(xf[:, 0, :]), r(tvpe_sb[:, 0, :]), start=True, stop=False)
        mm(vq_ps, r(xf[:, 1, :]), r(tvpe_sb[:, 1, :]), start=False, stop=False)
        mm(vq_ps, r(ones_row), r(tvpe_b_sb), start=False, stop=True)
        tqa = pa.tile([CH, P], f32, tag="tqa")
        nc.scalar.activation(tqa, vq_ps[:, D:D + P], AF.Tanh)
        aqa = pa.tile([CH, P], f32, tag="aqa")
        nc.scalar.activation(aqa, tqa, AF.Abs)
        kf = pa.tile([CH, 2 * P], f32, tag="kf")
        nc.scalar.activation(kf[:, 0:P], aqa, AF.Sin, scale=-PI, bias=halfpi)
        nc.scalar.activation(kf[:, P:2 * P], tqa, AF.Sin, scale=PI)
        km = pa.tile([CH, 1], f32, tag="km")
        dma(out=km, in_=t["kmask"][ci * CH:(ci + 1) * CH, :])
        kfm = pa.tile([CH, 2 * P], f32, tag="kfm")
        nc.vector.tensor_tensor(kfm, kf, km.broadcast_to([CH, 2 * P]),
                                OP.mult)
        v_sb = pa.tile([CH, D + 8], f32, tag="v_sb")
        nc.scalar.copy(v_sb[:, 0:D], vq_ps[:, 0:D])
        nc.vector.memset(v_sb[:, D:D + 8], 1.0)
        mm(S_ps, r(kfm), r(v_sb), start=(ci == 0), stop=(ci == NCH_B - 1))
    # S_h0 = S'[:, :D] + (sum kfm) x tv_b   (rank-1 bias fold)
    kfsum = own.tile([2 * P, 1], f32)
    nc.vector.tensor_copy(kfsum, S_ps[:, D:D + 1])
    S_tmp = own.tile([2 * P, D], f32)
    nc.vector.tensor_tensor(S_tmp, tv_b_bc64,
                            kfsum.broadcast_to([2 * P, D]), OP.mult)
    S_h0 = own.tile([2 * P, D], f32)
    nc.vector.tensor_tensor(S_h0, S_tmp, S_ps[:, 0:D], OP.add)
    S_h = [S_h0, S_h0]

    acc = own.tile([CH, 2, SEG], f32)
    nc.vector.memset(acc, 0.0)

    psA.release()
    psB = tc.alloc_tile_pool(name="psB", bufs=1, space="PSUM")

    # intra score blocks per half: (key chunk, local query lo, n, masked)
    HALF_BLOCKS = {
        0: [(0, 0, 2 * CH, True), (1, CH, CH, True)],
        1: [(0, 0, 2 * CH, False), (1, 0, 2 * CH, False),
            (2, 0, 2 * CH, True), (3, CH, CH, True)],
    }
    SH_IDX = {0: 0, 1: 0}

    # ---- refinement iterations, two half-segment pipelines ----
    for it in range(I):
        q = qA if it % 2 == 0 else qB
        qn = qB if it % 2 == 0 else qA

        w1k = pb.tile([CH, 2, 2 * D], f32, tag="w1k", bufs=2)
        dma(out=w1k, in_=t["w1g"][it, 0:2 * CH, :]
            .rearrange("(c p) m -> p c m", c=2))
        w1k2 = pb.tile([H, 2 * D], f32, tag="w1k2", bufs=2)
        dma(out=w1k2, in_=t["w1g"][it, 2 * CH:2 * CH + H, :])
        b1 = pb.tile([CH, 4], f32, tag="b1", bufs=2)
        dma(out=b1, in_=t["b1e_t"][it])
        w2k = pb.tile([CH, 4, D], f32, tag="w2k", bufs=2)
        dma(out=w2k, in_=t["w2"][it].rearrange("(c p) m -> p c m", c=4))
        b2 = pb.tile([CH, 2], f32, tag="b2", bufs=2)
        dma(out=b2, in_=t["b2_t"][it])
        if it < I - 1:
            gwk = pb.tile([CH, 4, D], f32, tag="gwk", bufs=2)
            dma(out=gwk, in_=t["gate_w"][it].rearrange("(c p) m -> p c m", c=4))
            gb = pb.tile([CH, 2], f32, tag="gb", bufs=2)
            dma(out=gb, in_=t["gb_t"][it])

        qfs_l = []
        for h in range(2):
            hsl = slice(h * HSEG, (h + 1) * HSEG)
            # Qf (feature-major) with 1/norm folded in
            if it > 0:
                qp_ps = psB.tile([P, HSEG], f32, tag="mix", bufs=2,
                                 name="qp_ps")
                mm(qp_ps, r(pe_w_sb[:, 0, :]), r(q[:, 0, hsl]),
                   start=True, stop=False)
                mm(qp_ps, r(pe_w_sb[:, 1, :]), r(q[:, 1, hsl]),
                   start=False, stop=True)
                tq_ = pb.tile([P, HSEG], f32, tag="tq")
                nc.scalar.activation(tq_, qp_ps, AF.Tanh, bias=pe_b_col_sb)
                aq_ = pb.tile([P, HSEG], f32, tag="aq")
                nc.scalar.activation(aq_, tq_, AF.Abs)
                qf = pb.tile([2 * P, HSEG], f32, tag="qf")
                nc.scalar.activation(qf[0:P, :], aq_, AF.Sin, scale=-PI,
                                     bias=halfpi[0:P, :])
                nc.scalar.activation(qf[P:2 * P, :], tq_, AF.Sin, scale=PI)
            else:
                qf = kff[:, hsl]
            qfs = pb.tile([2 * P, HSEG], f32, tag="qfs")
            nc.vector.tensor_mul(qfs, qf, inv_norm_sb[:, hsl])
            qfs_l.append(qfs)

        afm_full = pb.tile([H, SEG], f32, tag="afm")
        sqa_full = pb.tile([H, SEG], f32, tag="sqa")
        for h in range(2):
            hsl = slice(h * HSEG, (h + 1) * HSEG)
            # attention logits + tanh-softmax (token-major)
            z_ps = psB.tile([CH, 2, H], f32, tag="mix", bufs=2, name="z_ps")
            for c in range(2):
                sl = slice((2 * h + c) * CH, (2 * h + c + 1) * CH)
                mm(z_ps[:, c, :], r(q[:, 0, sl]), r(mq_w_sb[:, 0, :]),
                   start=True, stop=False)
                mm(z_ps[:, c, :], r(q[:, 1, sl]), r(mq_w_sb[:, 1, :]),
                   start=False, stop=True)
            zm = pb.tile([CH, 2], f32, tag="zm")
            nc.vector.tensor_reduce(zm, z_ps, AX, OP.max)
            zc = pb.tile([CH, 2, H], f32, tag="zc")
            nc.vector.tensor_tensor(zc, z_ps,
                                    zm.unsqueeze(-1).broadcast_to([CH, 2, H]),
                                    OP.subtract)
            nc.vector.tensor_tensor(
                zc, zc, mq_b_bc.unsqueeze(1).broadcast_to([CH, 2, H]), OP.add)
            th = pb.tile([CH, 2, H], f32, tag="th")
            nc.scalar.activation(th, zc, AF.Tanh, scale=0.5)
            num = pb.tile([CH, 2, H], f32, tag="num")
            nc.vector.tensor_scalar_add(num, th, 1.0)
            den = pb.tile([CH, 2, H], f32, tag="den")
            nc.vector.tensor_scalar(den, th, -1.0, 1.0, OP.mult, OP.add)
            rec = pb.tile([CH, 2, H], f32, tag="rec")
            nc.vector.reciprocal(rec, den)
            ex = pb.tile([CH, 2, H], f32, tag="ex")
            nc.vector.tensor_mul(ex, num, rec)
            es = pb.tile([CH, 2], f32, tag="es")
            nc.vector.tensor_reduce(es, ex, AX, OP.add)
            esr = pb.tile([CH, 2], f32, tag="esr")
            nc.vector.reciprocal(esr, es)
            at = pb.tile([CH, 2, H], f32, tag="at")
            nc.vector.tensor_tensor(at, ex,
                                    esr.unsqueeze(-1).broadcast_to([CH, 2, H]),
                                    OP.mult)
            for c in range(2):
                at_ps = psB.tile([H, CH], f32, tag="mix", bufs=2,
                                 name="at_ps")
                nc.tensor.transpose(at_ps, at[:, c, :], ident_sb)
                nc.vector.tensor_copy(
                    afm_full[:, (2 * h + c) * CH:(2 * h + c + 1) * CH],
                    at_ps)

        cn_full = pb.tile([CH, 2, SEG], f32, tag="cn")
        cna_full = pb.tile([H, SEG], f32, tag="cna")
        for h in range(2):
            hsl = slice(h * HSEG, (h + 1) * HSEG)
            qfs = qfs_l[h]
            # retrieval: inter (Qf@S) + intra masked quadratic
            r_ps = psB.tile([CH, 2, HSEG], f32, tag="r", bufs=2, name="r_ps")
            for dd in range(2):
                # single open accumulation group per PSUM bank: only the
                # first matmul may carry start=True
                mm(r_ps[:, dd, :], r(S_h[SH_IDX[h]][:, dd * CH:(dd + 1) * CH]),
                   r(qfs), start=(dd == 0), stop=False, skip_group_check=True)
            for bi_, (kc, lo, n, masked) in enumerate(HALF_BLOCKS[h]):
                qsl = slice(lo, lo + n)
                sc_ps = psB.tile([CH, 2 * CH], f32, tag="sc", bufs=2,
                                 name="sc_ps")
                mm(sc_ps[:, 0:n], r(kff[:, kc * CH:(kc + 1) * CH]),
                   r(qfs[:, qsl]), start=True, stop=True)
                sc_sb = pb.tile([CH, 2 * CH], f32, tag="sc_sb")
                if masked:
                    nc.vector.tensor_mul(sc_sb[:, 0:CH], sc_ps[:, 0:CH],
                                         tril_sb)
                else:
                    nc.vector.tensor_copy(sc_sb[:, 0:CH], sc_ps[:, 0:CH])
                if n > CH:
                    nc.vector.tensor_copy(sc_sb[:, CH:n], sc_ps[:, CH:n])
                last = bi_ == len(HALF_BLOCKS[h]) - 1
                for dd in range(2):
                    mm(r_ps[:, dd, qsl], r(vo[:, kc, dd * CH:(dd + 1) * CH]),
                       r(sc_sb[:, 0:n]), start=False,
                       stop=(last and dd == 1), skip_group_check=True)

            # retrieved -> SBUF (DVE) + squares (ACT, from PSUM, parallel)
            rt = pb.tile([CH, 2, HSEG], f32, tag="rt")
            sq = pb.tile([CH, 2, HSEG], f32, tag="sq")
            for dd in range(2):
                nc.vector.tensor_copy(rt[:, dd, :], r_ps[:, dd, :])
                nc.scalar.activation(sq[:, dd, :], r_ps[:, dd, :], AF.Square)
            nc.vector.tensor_mul(sqa_full[:, hsl], afm_full[:, hsl],
                                 afm_full[:, hsl])

            # LN stats over 264 features via ones-matmuls
            st1 = psB.tile([1, HSEG], f32, tag="mix", bufs=2, name="st1")
            mm(st1, r(oc264), r(rt[:, 0, :]), start=True, stop=False)
            mm(st1, r(oc264), r(rt[:, 1, :]), start=False, stop=False)
            mm(st1, r(oc264[0:H, :]), r(afm_full[:, hsl]), start=False,
               stop=True)
            st2 = psB.tile([1, HSEG], f32, tag="mix", bufs=2, name="st2")
            mm(st2, r(oc264), r(sq[:, 0, :]), start=True, stop=False)
            mm(st2, r(oc264), r(sq[:, 1, :]), start=False, stop=False)
            mm(st2, r(oc264[0:H, :]), r(sqa_full[:, hsl]), start=False,
               stop=True)
            m_sb = pb.tile([1, HSEG], f32, tag="m_sb")
            nc.vector.tensor_copy(m_sb, st1)
            msq = pb.tile([1, HSEG], f32, tag="msq")
            nc.vector.tensor_mul(msq, m_sb, m_sb)
            var = pb.tile([1, HSEG], f32, tag="var")
            nc.vector.tensor_tensor(var, st2, msq, OP.subtract)
            sd = pb.tile([1, HSEG], f32, tag="sd")
            nc.scalar.activation(sd, var, AF.Sqrt, bias=epsb[0:1, :])
            rstd = pb.tile([1, HSEG], f32, tag="rstd")
            nc.vector.reciprocal(rstd, sd)
            mr = pb.tile([1, HSEG], f32, tag="mr")
            nc.vector.tensor_mul(mr, m_sb, rstd)
            rbb = psB.tile([CH, 2, HSEG], f32, tag="mix", bufs=2, name="rbb")
            mm(rbb[:, 0, :], r(ones_row), r(rstd), start=True, stop=True,
               skip_group_check=True)
            mm(rbb[:, 1, :], r(ones_row), r(mr), start=False, stop=True,
               skip_group_check=True)
            rb_sb = pb.tile([CH, 2, HSEG], f32, tag="rb_sb")
            nc.vector.tensor_copy(rb_sb, rbb)

            for dd in range(2):
                nc.vector.tensor_mul(cn_full[:, dd, hsl], rt[:, dd, :],
                                     rb_sb[:, 0, :])
                nc.vector.tensor_tensor(cn_full[:, dd, hsl],
                                        cn_full[:, dd, hsl],
                                        rb_sb[:, 1, :], OP.subtract)
            nc.vector.tensor_mul(cna_full[:, hsl], afm_full[:, hsl],
                                 rb_sb[0:H, 0, :])
            nc.vector.tensor_tensor(cna_full[:, hsl], cna_full[:, hsl],
                                    rb_sb[0:H, 1, :], OP.subtract)

        # ---- joint full-width MLP: w1 + gelu ----
        hh = pb.tile([CH, 4, SEG], f32, tag="h")
        for o in range(4):
            osl = slice(o * CH, (o + 1) * CH)
            h_ps = psB.tile([CH, SEG], f32, tag="h", bufs=2, name="h_ps")
            mm(h_ps, r(w1k[:, 0, osl]), r(cn_full[:, 0, :]),
               start=True, stop=False)
            mm(h_ps, r(w1k[:, 1, osl]), r(cn_full[:, 1, :]),
               start=False, stop=False)
            mm(h_ps, r(w1k2[:, osl]), r(cna_full), start=False, stop=True)
            nc.scalar.activation(hh[:, o, :], h_ps, AF.Gelu,
                                 bias=b1[:, o:o + 1])

        # w2 (+b2), accumulate
        rf = pb.tile([CH, 2, SEG], f32, tag="rf")
        for m_ in range(2):
            msl = slice(m_ * CH, (m_ + 1) * CH)
            rf_ps = psB.tile([CH, SEG], f32, tag="h", bufs=2, name="rf_ps")
            for k in range(4):
                mm(rf_ps, r(w2k[:, k, msl]), r(hh[:, k, :]),
                   start=(k == 0), stop=(k == 3))
            nc.scalar.activation(rf[:, m_, :], rf_ps, AF.Identity,
                                 bias=b2[:, m_:m_ + 1])
            nc.vector.tensor_add(acc[:, m_, :], acc[:, m_, :], rf[:, m_, :])

        # gate -> next query (skipped on last iteration)
        if it < I - 1:
            for m_ in range(2):
                msl = slice(m_ * CH, (m_ + 1) * CH)
                g_ps = psB.tile([CH, SEG], f32, tag="h", bufs=2, name="g_ps")
                for k in range(4):
                    rhs = q[:, k, :] if k < 2 else rf[:, k - 2, :]
                    mm(g_ps, r(gwk[:, k, msl]), r(rhs),
                       start=(k == 0), stop=(k == 3))
                gd = pb.tile([CH, SEG], f32, tag="gd")
                nc.scalar.activation(gd, g_ps, AF.Tanh,
                                     bias=gb[:, m_:m_ + 1])
                nc.vector.tensor_add(qn[:, m_, :], q[:, m_, :], gd)

    # ---- final LN(acc) @ wog + (x + bo) ----
    for h in range(2):
        hsl = slice(h * HSEG, (h + 1) * HSEG)
        sqf = pb.tile([CH, 2, HSEG], f32, tag="sq")
        for dd in range(2):
            nc.vector.tensor_mul(sqf[:, dd, :], acc[:, dd, hsl],
                                 acc[:, dd, hsl])
        st1f = psB.tile([1, HSEG], f32, tag="mix", bufs=2, name="st1f")
        mm(st1f, r(oc256), r(acc[:, 0, hsl]), start=True, stop=False)
        mm(st1f, r(oc256), r(acc[:, 1, hsl]), start=False, stop=True)
        st2f = psB.tile([1, HSEG], f32, tag="mix", bufs=2, name="st2f")
        mm(st2f, r(oc256), r(sqf[:, 0, :]), start=True, stop=False)
        mm(st2f, r(oc256), r(sqf[:, 1, :]), start=False, stop=True)
        mf = pb.tile([1, HSEG], f32, tag="m_sb")
        nc.vector.tensor_copy(mf, st1f)
        msqf = pb.tile([1, HSEG], f32, tag="msq")
        nc.vector.tensor_mul(msqf, mf, mf)
        varf = pb.tile([1, HSEG], f32, tag="var")
        nc.vector.tensor_tensor(varf, st2f, msqf, OP.subtract)
        sdf = pb.tile([1, HSEG], f32, tag="sd")
        nc.scalar.activation(sdf, varf, AF.Sqrt, bias=epsb[0:1, :])
        rstdf = pb.tile([1, HSEG], f32, tag="rstd")
        nc.vector.reciprocal(rstdf, sdf)
        mrf = pb.tile([1, HSEG], f32, tag="mr")
        nc.vector.tensor_mul(mrf, mf, rstdf)
        rbb = psB.tile([CH, 2, HSEG], f32, tag="mix", bufs=2, name="rbbf")
        mm(rbb[:, 0, :], r(ones_row), r(rstdf), start=True, stop=True,
           skip_group_check=True)
        mm(rbb[:, 1, :], r(ones_row), r(mrf), start=True, stop=True,
           skip_group_check=True)
        rbf_sb = pb.tile([CH, 2, HSEG], f32, tag="rb_sb")
        nc.vector.tensor_copy(rbf_sb, rbb)
        cnf = pb.tile([CH, 2, HSEG], f32, tag="cn")
        for dd in range(2):
            nc.vector.tensor_mul(cnf[:, dd, :], acc[:, dd, hsl],
                                 rbf_sb[:, 0, :])
            nc.vector.tensor_tensor(cnf[:, dd, :], cnf[:, dd, :],
                                    rbf_sb[:, 1, :], OP.subtract)
        for c in range(2):
            cc = 2 * h + c
            sl = slice(c * CH, (c + 1) * CH)
            o_ps = psB.tile([CH, D], f32, tag="sc", bufs=2, name="o_ps")
            mm(o_ps, r(cnf[:, 0, sl]), r(wog_sb[:, 0, :]),
               start=True, stop=False)
            mm(o_ps, r(cnf[:, 1, sl]), r(wog_sb[:, 1, :]),
               start=False, stop=True)
            yt = pb.tile([CH, D], f32, tag="yt")
            nc.vector.tensor_add(yt, o_ps, x_tm_sb[:, cc, :])
            if not os.environ.get("DEBUG_RT"):
                dma(out=t["y"][cc * CH:(cc + 1) * CH, :], in_=yt)

    for pool in (psB, pb, pa, own, consts):
        pool.release()


def _prep_inputs(inputs):
    """Host-side parameter folding + per-core input maps."""
    f = lambda a: np.ascontiguousarray(np.asarray(a, dtype=np.float32))
    x = f(inputs["x"])
    pe_w, pe_b = f(inputs["pe_w"]), f(inputs["pe_b"])
    tv_w, tv_b = f(inputs["tv_w"]), f(inputs["tv_b"])
    mq_w, mq_b = f(inputs["mq_w"]), f(inputs["mq_b"])
    ln_g, ln_b = f(inputs["ref_ln_g"]), f(inputs["ref_ln_b"])
    w1, b1 = f(inputs["ref_w1"]), f(inputs["ref_b1"])
    w2, b2 = f(inputs["ref_w2"]), f(inputs["ref_b2"])
    gw, gb = f(inputs["gate_w"]), f(inputs["gate_b"])
    og, ob = f(inputs["out_ln_g"]), f(inputs["out_ln_b"])
    ow, obias = f(inputs["out_w"]), f(inputs["out_b"])

    w1g = ln_g[:, :, None] * w1
    b1e = b1 + np.einsum("if,ifo->io", ln_b, w1)
    wog = og[:, None] * ow
    boe = obias + ob @ ow

    shared = {
        "pe_w": pe_w, "pe_b_row": pe_b[None, :], "pe_b_col": pe_b[:, None],
        "tv_w": tv_w, "tv_b_row": tv_b[None, :],
        "tvpe_w": np.ascontiguousarray(np.concatenate([tv_w, pe_w], axis=1)),
        "tvpe_b": np.ascontiguousarray(np.concatenate([np.zeros_like(tv_b), pe_b])[None, :]),
        "mq_w": mq_w, "mq_b_row": mq_b[None, :],
        "w1g": w1g,
        "b1e_t": np.ascontiguousarray(
            b1e.reshape(I, 4, CH).transpose(0, 2, 1)),
        "w2": w2,
        "b2_t": np.ascontiguousarray(b2.reshape(I, 2, CH).transpose(0, 2, 1)),
        "gate_w": gw,
        "gb_t": np.ascontiguousarray(gb.reshape(I, 2, CH).transpose(0, 2, 1)),
        "wog": wog,
        "ident": np.eye(CH, dtype=np.float32),
        "tril": np.triu(np.ones((CH, CH), dtype=np.float32)),
    }
    shared = {k: np.ascontiguousarray(v) for k, v in shared.items()}

    in_maps = []
    for core in range(NCORES):
        b, pos = divmod(core, NCORES // B)
        s0 = pos * SEG
        xb_t = np.ascontiguousarray(x[b].T)  # (D, L)
        km = (np.arange(L) < s0).astype(np.float32)[:, None]
        gl = np.arange(s0, s0 + SEG, dtype=np.float64)
        invn = (1.0 / (np.sqrt(gl + 1.0) * math.sqrt(P))).astype(np.float32)
        m = dict(shared)
        m["x_pref_fm"] = xb_t
        m["kmask"] = km
        m["x_own_fm"] = np.ascontiguousarray(xb_t[:, s0:s0 + SEG])
        m["x_own_tm"] = np.ascontiguousarray(x[b, s0:s0 + SEG, :]
                                             + boe[None, :])
        m["inv_norm"] = np.ascontiguousarray(
            np.broadcast_to(invn[None, :], (2 * P, SEG)))
        in_maps.append(m)
    return in_maps


def kernel(**inputs):
    from concourse.bass_utils import run_bass_kernel_spmd

    if "nc" not in _CACHE:
        _CACHE["nc"] = _build_program()
    nc = _CACHE["nc"]
    in_maps = _prep_inputs(inputs)
    res = run_bass_kernel_spmd(nc, in_maps, core_ids=list(range(NCORES)))
    out = np.empty((B, L, D), dtype=np.float32)
    for core in range(NCORES):
        b, pos = divmod(core, NCORES // B)
        s0 = pos * SEG
        out[b, s0:s0 + SEG, :] = res.results[core]["y"]
    return out



# revision 13
# speedup vs baseline: 1.2231x; 1.2231x over previous
"""Trainium2 Bass kernel for nn_AttentionGuidedIterativeBlock.

Math reformulation: the (B,L,P,D) phasor cumsum + retrieval is causal linear
attention with feature map Kf = [cos(phases), sin(phases)] (2P=64 dims):

    retrieved[l] = (sum_{l'<=l} (Qf[l].Kf[l']) * V[l']) / (sqrt(l+1)*sqrt(P))

Sharding: 8 cores x 512 tokens (cores 0-3 batch 0, 4-7 batch 1).  Each core
rebuilds the prefix state S = Kf^T @ V_aug over its batch prefix from a
per-core slot-permuted copy of x (12 prefix slots, zero-padded + masked via a
per-slot km scalar, then the 4 own chunks), so the program is uniform across
cores while own-segment K/V prep and prefix-state accumulation share the same
matmuls.  Everything runs in bf16 on the PE (fp32 PSUM accumulate); LayerNorm
mean-subtraction is folded into w1/wog as a rank-1 matmul (-mr x colsum(W)),
gelu(exact erf) is replaced by the sigmoid approximation expressed as
silu(1.702x)/1.702 so every activation function used stays inside one
activation-table set except nothing -- zero table swaps except none.
"""

import math
import os

import numpy as np
import ml_dtypes

D, P, I, H = 256, 32, 3, 8
B, L = 2, 2048
NCORES = 8
SEG = 512          # tokens per core
CH = 128           # chunk (tile partition) size
NSLOT = 16         # 12 prefix slots + 4 own chunks
NPREF = 12
HSEG = 256
PI = math.pi
EPS = 1e-5
GSC = 1.702        # sigmoid-gelu: gelu(x) ~= silu(GSC*x)/GSC

_CACHE = {}

# ---- shared-blob column offsets (bf16, 128 partitions) ----
# blobA (early consts): tvpe (2,288), ident(128), tril(128), oc264, oc256,
#   onescol, onesrow(row0,128), pe_b_row(row0,32)
A_TVPE = 0
A_ID = 576
A_TRIL = 704
A_OC264 = 832
A_OC256 = 833
A_ONEC = 834
A_ONER = 835          # row0 cols 835:963 = 1.0
A_PEBR = 963          # row0 cols 963:995
A_COLS = 995

# blobB (weights): pe_w (2,32), mq_w (2,8), wog (2,2,128), w1k I*(2,512),
#   w2k I*(4,2,128), gwk 2*(4,2,128), w1k2 I*(8rows,512), w1sum_neg I*(row0,512),
#   wogsum_neg (row0, 256)
B_PEW = 0
B_MQW = 64
B_WOG = 80
B_W1K = 592
B_W2K = B_W1K + I * 1024          # 3664
B_GWK = B_W2K + I * 1024          # 6736
B_W1K2 = B_GWK + 2 * 1024         # 8784 (rows 0:8)
B_W1SUM = B_W1K2 + I * 512        # 10320 (row0)
B_WOGSUM = B_W1SUM + I * 512      # 11856 (row0)
B_COLS = B_WOGSUM + 256           # 12112

# blobF (fp32 biases): pe_b_col(32,1), mq_bh_col(8,1), b1s I*(128,4),
#   b2 I*(128,2), gb 2*(128,2)
F_PEB = 0
F_MQB = 1
F_B1 = 2
F_B2 = F_B1 + I * 4       # 14
F_GB = F_B2 + I * 2       # 20
F_COLS = F_GB + 4         # 24

# blobC (per-core): x_perm (2,2048), invn (rows0:64, 512), km (1? full col, 16)
C_X = 0
C_INVN = 4096
C_KM = 4608
C_COLS = 4624


def _patch_walrus_passes():
    import concourse.bass_utils as bu
    if getattr(bu, "_nv_patched", False):
        return
    orig = bu.run_command

    def patched(cmd, cwd=None, **kw):
        cmd = list(cmd)
        if "--pass" in cmd:
            i = cmd.index("--pass")
            cmd[i + 1] = cmd[i + 1].replace("birverifier,", "")
        return orig(cmd, cwd=cwd, **kw)

    bu.run_command = patched
    bu._nv_patched = True


def _build_program(split=True):
    _patch_walrus_passes()
    import concourse.bass as bass
    import concourse.tile as tile
    from concourse import mybir

    f32 = mybir.dt.float32

    nc = bass.Bass("TRN2", target_bir_lowering=False, debug=False,
                   num_devices=NCORES)

    def din(name, shape, dt):
        return nc.dram_tensor(name, shape, dt, kind="ExternalInput").ap()

    bf16 = mybir.dt.bfloat16
    t = {}
    t["blobA"] = din("blobA", (CH, A_COLS), bf16)
    t["blobB"] = din("blobB", (CH, B_COLS), bf16)
    t["blobC"] = din("blobC", (CH, C_COLS), bf16)
    t["blobF"] = din("blobF", (CH, F_COLS), f32)
    t["tvb"] = din("tvb", (1, D), f32)
    t["x_fm"] = din("x_fm", (CH, 2, SEG), f32)
    t["y"] = nc.dram_tensor("y", (CH, 2, SEG), f32, kind="ExternalOutput").ap()

    with tile.TileContext(nc) as tc:
        _body(tc, nc, t, f32, bf16, bass, mybir)
    if split:
        _split_waits(nc, mybir)
    return nc


def _split_waits(nc, mybir, cap=1):
    """Move excess sync-waits onto preceding same-engine NOPs (this walrus
    build allows one wait slot per instruction)."""
    for fn in nc.m.functions:
        for blk in fn.blocks:
            out = []
            for ins in blk.instructions:
                si = ins.sync_info
                if si is not None and len(si.on_wait) > cap:
                    waits = list(si.on_wait)
                    extra, keep = waits[:-cap], waits[-cap:]
                    for j, w in enumerate(extra):
                        nop = mybir.InstNoOp(name=f"{ins.name}_wsplit{j}",
                                             ins=[], outs=[])
                        nop.engine = ins.engine
                        nop.sync_info = mybir.SyncInfo(on_wait=[w],
                                                       on_update=[])
                        out.append(nop)
                    ins.sync_info = mybir.SyncInfo(on_wait=keep,
                                                   on_update=si.on_update)
                out.append(ins)
            blk.instructions = out


def _body(tc, nc, t, f32, bf16, bass, mybir):
    from concourse.alu_op_type import AluOpType as OP

    AF = mybir.ActivationFunctionType

    lp = nc.allow_low_precision(reason="bf16 kernel; tolerance 2e-2 validated")
    lp.__enter__()
    consts = tc.alloc_tile_pool(name="consts", bufs=1)
    own = tc.alloc_tile_pool(name="own", bufs=1)
    pa = tc.alloc_tile_pool(name="pa", bufs=2)
    pb = tc.alloc_tile_pool(name="pb", bufs=2)

    dma = nc.sync.dma_start
    mm = nc.tensor.matmul

    # ---- constant blobs (6 input DMAs total) ----
    cA = consts.tile([CH, A_COLS], bf16)
    dma(out=cA, in_=t["blobA"])
    cC = consts.tile([CH, C_COLS], bf16)
    dma(out=cC, in_=t["blobC"])
    cB = consts.tile([CH, B_COLS], bf16)
    dma(out=cB, in_=t["blobB"])
    cF = consts.tile([CH, F_COLS], f32)
    dma(out=cF, in_=t["blobF"])
    tvb_bc = consts.tile([2 * P, D], f32)
    dma(out=tvb_bc, in_=t["tvb"].to_broadcast((2 * P, D)))
    xfm_sb = consts.tile([CH, 2, SEG], f32)
    dma(out=xfm_sb, in_=t["x_fm"])

    def xslot(kh, s):
        return cC[:, C_X + kh * 2048 + s * CH: C_X + kh * 2048 + (s + 1) * CH]

    tvpe = lambda kh: cA[:, A_TVPE + kh * 288: A_TVPE + (kh + 1) * 288]
    ident = cA[:, A_ID:A_ID + CH]
    tril = cA[:, A_TRIL:A_TRIL + CH]
    oc264 = cA[:, A_OC264:A_OC264 + 1]
    oc256 = cA[:, A_OC256:A_OC256 + 1]
    onesr = cA[0:1, A_ONER:A_ONER + CH]
    pebr = cA[0:1, A_PEBR:A_PEBR + P]
    invn = cC[0:2 * P, C_INVN:C_INVN + SEG]
    pe_b_col = cF[0:P, F_PEB:F_PEB + 1]
    mq_bh_col = cF[0:H, F_MQB:F_MQB + 1]
    halfpi = consts.tile([CH, 1], f32)
    nc.vector.memset(halfpi, PI / 2)
    i32 = mybir.dt.int32
    qmagic = consts.tile([1, SEG], i32)
    nc.vector.memset(qmagic, 0x5F3759DF)

    def ln_rstd(st2_psum, m_b, tagp):
        # rstd = 1/sqrt(st2 - m^2 + eps), DVE-only (Quake seed + 1 Newton)
        msq = pb.tile([1, SEG], f32, tag=tagp + "msq")
        nc.scalar.activation(msq, m_b, AF.Square)
        msqe = pb.tile([1, SEG], f32, tag=tagp + "msqe")
        nc.vector.tensor_scalar_add(msqe, msq, -EPS)
        var = pb.tile([1, SEG], f32, tag=tagp + "var")
        nc.vector.tensor_tensor(var, st2_psum, msqe, OP.subtract)
        sh = pb.tile([1, SEG], i32, tag=tagp + "sh")
        nc.vector.tensor_scalar(sh, var.bitcast(i32), 1, None,
                                OP.logical_shift_right)
        si = pb.tile([1, SEG], i32, tag=tagp + "si")
        nc.vector.tensor_tensor(si, qmagic, sh, OP.subtract)
        y = si.bitcast(f32)
        y2 = pb.tile([1, SEG], f32, tag=tagp + "y2")
        nc.scalar.activation(y2, y, AF.Square)
        tn = pb.tile([1, SEG], f32, tag=tagp + "tn")
        nc.vector.tensor_tensor(tn, y2, var, OP.mult)
        un = pb.tile([1, SEG], f32, tag=tagp + "un")
        nc.vector.tensor_scalar(un, tn, -0.5, 1.5, OP.mult, OP.add)
        rstd = pb.tile([1, SEG], bf16, tag=tagp + "rstd")
        nc.vector.tensor_tensor(rstd, y, un, OP.mult)
        return rstd

    pe_w = lambda kh: cB[:, B_PEW + kh * P: B_PEW + (kh + 1) * P]
    mq_w = lambda kh: cB[:, B_MQW + kh * H: B_MQW + (kh + 1) * H]
    wog = lambda kh, mh: cB[:, B_WOG + (kh * 2 + mh) * CH:
                            B_WOG + (kh * 2 + mh + 1) * CH]
    w1k = lambda it, kh, o: cB[:, B_W1K + it * 1024 + kh * 512 + o * CH:
                               B_W1K + it * 1024 + kh * 512 + (o + 1) * CH]
    w2k = lambda it, kh, mh: cB[:, B_W2K + it * 1024 + (kh * 2 + mh) * CH:
                                B_W2K + it * 1024 + (kh * 2 + mh + 1) * CH]
    gwk = lambda it, kh, mh: cB[:, B_GWK + it * 1024 + (kh * 2 + mh) * CH:
                                B_GWK + it * 1024 + (kh * 2 + mh + 1) * CH]
    w1k2 = lambda it, o: cB[0:H, B_W1K2 + it * 512 + o * CH:
                            B_W1K2 + it * 512 + (o + 1) * CH]
    w1sum = lambda it, o: cB[0:1, B_W1SUM + it * 512 + o * CH:
                             B_W1SUM + it * 512 + (o + 1) * CH]
    wogsum = lambda mh: cB[0:1, B_WOGSUM + mh * CH:B_WOGSUM + (mh + 1) * CH]

    # =========== phase A: prefix state + own K/V (unified slot loop) =======
    psA = tc.alloc_tile_pool(name="psA", bufs=1, space="PSUM")

    S_ps = psA.tile([2 * P, D + 8], f32, tag="S")
    own_kf = own.tile([CH, 4, 2 * P], bf16)    # own kf, token-major, unmasked
    v_own = own.tile([CH, 4, D + 8], bf16)     # own V_aug, token-major

    for g in range(4):
        vq_ps = psA.tile([CH, 4, 512], f32, tag="vq", bufs=1,
                         name=f"vq{g}")
        for c in range(4):
            s = 4 * g + c
            mm(vq_ps[:, c, 0:D + P], xslot(0, s), tvpe(0),
               start=True, stop=False, skip_group_check=True)
            mm(vq_ps[:, c, 0:D + P], xslot(1, s), tvpe(1),
               start=False, stop=False, skip_group_check=True)
            mm(vq_ps[:, c, D:D + P], onesr, pebr, start=False, stop=True,
               skip_group_check=True)
        tq = pa.tile([CH, 4, P], bf16, tag="tq")
        nc.scalar.activation(tq, vq_ps[:, :, D:D + P], AF.Tanh)
        aq = pa.tile([CH, 4, P], bf16, tag="aq")
        nc.scalar.activation(aq, tq, AF.Abs)
        kfg = own_kf if g == 3 else pa.tile([CH, 4, 2 * P], bf16, tag="kf")
        nc.scalar.activation(kfg[:, :, 0:P], aq, AF.Sin, scale=-PI,
                             bias=halfpi)
        nc.scalar.activation(kfg[:, :, P:2 * P], tq, AF.Sin, scale=PI)
        vdst = v_own if g == 3 else pa.tile([CH, 4, D + 8], bf16, tag="vsb")
        nc.vector.tensor_copy(vdst[:, :, 0:D], vq_ps[:, :, 0:D])
        nc.vector.memset(vdst[:, :, D:D + 8], 1.0)
        if g < 3:
            kfm = pa.tile([CH, 4, 2 * P], bf16, tag="kfm")
            nc.vector.tensor_tensor(
                kfm, kfg,
                cC[:, C_KM + 4 * g:C_KM + 4 * g + 4].unsqueeze(-1)
                .broadcast_to([CH, 4, 2 * P]), OP.mult)
            for c in range(4):
                s = 4 * g + c
                mm(S_ps, kfm[:, c, :], vdst[:, c, :],
                   start=(s == 0), stop=(s == NPREF - 1),
                   skip_group_check=True)

    # S_h0 = prefix state (+ rank-1 tv_b fold);  S_h1 adds own chunks 0,1
    S_h = []
    for hi in range(2):
        if hi == 1:
            for c in range(2):
                mm(S_ps, own_kf[:, c, :], v_own[:, c, :],
                   start=False, stop=(c == 1), skip_group_check=True)
        kfsum = own.tile([2 * P, 1], f32, tag=f"kfsum{hi}")
        nc.vector.tensor_copy(kfsum, S_ps[:, D:D + 1])
        st = own.tile([2 * P, D], f32, tag=f"S_tmp{hi}")
        nc.vector.tensor_tensor(
            st, tvb_bc, kfsum.broadcast_to([2 * P, D]), OP.mult)
        sh = own.tile([2 * P, D], bf16, tag=f"S_h{hi}")
        nc.vector.tensor_tensor(sh, st, S_ps[:, 0:D], OP.add)
        S_h.append(sh)

    # kff: own kf feature-major (for intra scores + iteration-0 Qf)
    kff = own.tile([2 * P, SEG], bf16)
    for c in range(4):
        tr_ps = psA.tile([2 * P, CH], bf16, tag="tr", bufs=2, name="tr_ps")
        nc.tensor.transpose(tr_ps, own_kf[:, c, :], ident)
        nc.vector.tensor_copy(kff[:, c * CH:(c + 1) * CH], tr_ps)

    acc = own.tile([CH, 2, SEG], bf16)
    nc.vector.memset(acc, 0.0)
    qA = own.tile([CH, 2, SEG], bf16)
    qB = own.tile([CH, 2, SEG], bf16)
    # initial query = own x (bf16, feature-major) = slots 12-15 of x_perm
    for kh in range(2):
        nc.vector.tensor_copy(
            qA[:, kh, :],
            cC[:, C_X + kh * 2048 + NPREF * CH:C_X + kh * 2048 + NSLOT * CH])

    psA.release()
    psB = tc.alloc_tile_pool(name="psB", bufs=1, space="PSUM")

    # intra score blocks per half: (key chunk, local query lo, n)
    HALF_BLOCKS = {0: [(0, 0, 2 * CH), (1, CH, CH)],
                   1: [(2, 0, 2 * CH), (3, CH, CH)]}

    # =========== refinement iterations (full-width, feature-major) ========
    for it in range(I):
        q = qA if it % 2 == 0 else qB
        qn = qB if it % 2 == 0 else qA

        # Qf (feature-major), with 1/norm folded
        if it > 0:
            qp_ps = psB.tile([P, SEG], f32, tag="mix", bufs=2, name="qp_ps")
            mm(qp_ps, pe_w(0), q[:, 0, :], start=True, stop=False)
            mm(qp_ps, pe_w(1), q[:, 1, :], start=False, stop=True)
            tq_ = pb.tile([P, SEG], bf16, tag="tq")
            nc.scalar.activation(tq_, qp_ps, AF.Tanh, bias=pe_b_col)
            aq_ = pb.tile([P, SEG], bf16, tag="aq")
            nc.scalar.activation(aq_, tq_, AF.Abs)
            qf = pb.tile([2 * P, SEG], bf16, tag="qf")
            nc.scalar.activation(qf[0:P, :], aq_, AF.Sin, scale=-PI,
                                 bias=halfpi[0:P, :])
            nc.scalar.activation(qf[P:2 * P, :], tq_, AF.Sin, scale=PI)
        else:
            qf = kff
        qfs = pb.tile([2 * P, SEG], bf16, tag="qfs")
        nc.vector.tensor_tensor(qfs, qf, invn, OP.mult)

        # attention, feature-major tanh-softmax
        z_ps = psB.tile([H, SEG], f32, tag="mix", bufs=2, name="z_ps")
        mm(z_ps, mq_w(0), q[:, 0, :], start=True, stop=False)
        mm(z_ps, mq_w(1), q[:, 1, :], start=False, stop=True)
        th = pb.tile([H, SEG], bf16, tag="th")
        nc.scalar.activation(th, z_ps, AF.Tanh, scale=0.5, bias=mq_bh_col)
        num = pb.tile([H, SEG], bf16, tag="num")
        nc.vector.tensor_scalar_add(num, th, 1.0)
        den = pb.tile([H, SEG], bf16, tag="den")
        nc.vector.tensor_scalar(den, th, -1.0, 1.0, OP.mult, OP.add)
        rec = pb.tile([H, SEG], bf16, tag="rec")
        nc.vector.reciprocal(rec, den)
        ex = pb.tile([H, SEG], bf16, tag="ex")
        nc.vector.tensor_tensor(ex, num, rec, OP.mult)
        es_ps = psB.tile([1, SEG], f32, tag="mix", bufs=2, name="es_ps")
        mm(es_ps, cA[0:H, A_ONEC:A_ONEC + 1], ex, start=True, stop=True)
        esr = pb.tile([1, SEG], bf16, tag="esr")
        nc.vector.reciprocal(esr, es_ps)
        esb_ps = psB.tile([H, SEG], f32, tag="mix", bufs=2, name="esb_ps")
        mm(esb_ps, onesr[0:1, 0:H], esr, start=True, stop=True)
        at = pb.tile([H, SEG], bf16, tag="at")
        nc.vector.tensor_tensor(at, ex, esb_ps, OP.mult)
        sqa = pb.tile([H, SEG], bf16, tag="sqa")
        nc.scalar.activation(sqa, at, AF.Square)

        # retrieval: inter (Qf@S_h) + intra masked quadratic
        r_ps = psB.tile([CH, 2, SEG], f32, tag="r", bufs=1, name="r_ps")
        for h in range(2):
            hsl = slice(h * HSEG, (h + 1) * HSEG)
            for dd in range(2):
                mm(r_ps[:, dd, hsl], S_h[h][:, dd * CH:(dd + 1) * CH],
                   qfs[:, hsl], start=True, stop=False,
                   skip_group_check=True)
            for bi, (kc, lo, n) in enumerate(HALF_BLOCKS[h]):
                qsl = slice(h * HSEG + lo, h * HSEG + lo + n)
                sc_ps = psB.tile([CH, 2 * CH], f32, tag="sc", bufs=1,
                                 name="sc_ps")
                mm(sc_ps[:, 0:n], kff[:, kc * CH:(kc + 1) * CH],
                   qfs[:, qsl], start=True, stop=True)
                sc_sb = pb.tile([CH, 2 * CH], bf16, tag="sc_sb")
                nc.vector.tensor_tensor(sc_sb[:, 0:CH], sc_ps[:, 0:CH],
                                        tril, OP.mult)
                if n > CH:
                    nc.vector.tensor_copy(sc_sb[:, CH:n], sc_ps[:, CH:n])
                last = bi == len(HALF_BLOCKS[h]) - 1
                for dd in range(2):
                    mm(r_ps[:, dd, qsl], v_own[:, kc, dd * CH:(dd + 1) * CH],
                       sc_sb[:, 0:n], start=False,
                       stop=(last and dd == 1), skip_group_check=True)

        rt = pb.tile([CH, 2, SEG], bf16, tag="rt")
        nc.vector.tensor_copy(rt, r_ps)
        sq = pb.tile([CH, 2, SEG], bf16, tag="sq")
        nc.scalar.activation(sq, r_ps, AF.Square)

        # LN stats over 264 features via ones-matmuls (attn mean == 1/264)
        st_ps = psB.tile([33, SEG], f32, tag="st", bufs=1, name="st_ps")
        mm(st_ps[0:1, :], oc264, rt[:, 0, :], start=True, stop=False,
           skip_group_check=True)
        mm(st_ps[0:1, :], oc264, rt[:, 1, :], start=False, stop=True,
           skip_group_check=True)
        mm(st_ps[32:33, :], oc264, sq[:, 0, :], start=True, stop=False,
           skip_group_check=True)
        mm(st_ps[32:33, :], oc264, sq[:, 1, :], start=False, stop=False,
           skip_group_check=True)
        mm(st_ps[32:33, :], oc264[0:H, :], sqa, start=False, stop=True,
           skip_group_check=True)
        m_b = pb.tile([1, SEG], bf16, tag="m_b")
        nc.vector.tensor_scalar_add(m_b, st_ps[0:1, :], 1.0 / (D + H))
        rstd = ln_rstd(st_ps[32:33, :], m_b, "l")
        mr = pb.tile([1, SEG], bf16, tag="mr")
        nc.vector.tensor_tensor(mr, m_b, rstd, OP.mult)
        rbb = psB.tile([CH, SEG], f32, tag="mix", bufs=2, name="rbb")
        mm(rbb, onesr, rstd, start=True, stop=True)

        # cn = c * rstd   (mean folded into w1 via -mr x colsum(w1))
        cn = pb.tile([CH, 2, SEG], bf16, tag="cn")
        for dd in range(2):
            nc.vector.tensor_tensor(cn[:, dd, :], rt[:, dd, :], rbb, OP.mult)
        cna = pb.tile([H, SEG], bf16, tag="cna")
        nc.vector.tensor_tensor(cna, at, rbb[0:H, :], OP.mult)

        # joint MLP: w1 (+rank-1 mean fold) -> silu -> w2 (+b2)
        hh = pb.tile([CH, 4, SEG], bf16, tag="h")
        for o in range(4):
            h_ps = psB.tile([CH, SEG], f32, tag="h", bufs=2, name="h_ps")
            mm(h_ps, w1k(it, 0, o), cn[:, 0, :], start=True, stop=False)
            mm(h_ps, w1k(it, 1, o), cn[:, 1, :], start=False, stop=False)
            mm(h_ps, w1k2(it, o), cna, start=False, stop=False)
            mm(h_ps, w1sum(it, o), mr, start=False, stop=True)
            nc.scalar.activation(hh[:, o, :], h_ps, AF.Silu, scale=GSC,
                                 bias=cF[:, F_B1 + it * 4 + o:
                                         F_B1 + it * 4 + o + 1])

        rf = pb.tile([CH, 2, SEG], bf16, tag="rf")
        for mh in range(2):
            rf_ps = psB.tile([CH, SEG], f32, tag="h", bufs=2, name="rf_ps")
            for kh in range(4):
                mm(rf_ps, w2k(it, kh, mh), hh[:, kh, :],
                   start=(kh == 0), stop=(kh == 3))
            nc.scalar.activation(rf[:, mh, :], rf_ps, AF.Identity,
                                 bias=cF[:, F_B2 + it * 2 + mh:
                                         F_B2 + it * 2 + mh + 1])
        nc.vector.tensor_tensor(acc, acc, rf, OP.add)

        # gate -> next query (skipped on last iteration)
        if it < I - 1:
            gd = pb.tile([CH, 2, SEG], bf16, tag="gd")
            for mh in range(2):
                g_ps = psB.tile([CH, SEG], f32, tag="h", bufs=2, name="g_ps")
                for kh in range(4):
                    rhs = q[:, kh, :] if kh < 2 else rf[:, kh - 2, :]
                    mm(g_ps, gwk(it, kh, mh), rhs,
                       start=(kh == 0), stop=(kh == 3))
                nc.scalar.activation(gd[:, mh, :], g_ps, AF.Tanh,
                                     bias=cF[:, F_GB + it * 2 + mh:
                                             F_GB + it * 2 + mh + 1])
            nc.vector.tensor_tensor(qn, q, gd, OP.add)

    # =========== final LN(acc) @ wog + x (+boe) ===========
    sqf = pb.tile([CH, 2, SEG], bf16, tag="sq")
    nc.scalar.activation(sqf, acc, AF.Square)
    stf = psB.tile([33, SEG], f32, tag="st", bufs=1, name="stf")
    mm(stf[0:1, :], oc256, acc[:, 0, :], start=True, stop=False,
       skip_group_check=True)
    mm(stf[0:1, :], oc256, acc[:, 1, :], start=False, stop=True,
       skip_group_check=True)
    mm(stf[32:33, :], oc256, sqf[:, 0, :], start=True, stop=False,
       skip_group_check=True)
    mm(stf[32:33, :], oc256, sqf[:, 1, :], start=False, stop=True,
       skip_group_check=True)
    m_f = pb.tile([1, SEG], bf16, tag="m_b")
    nc.vector.tensor_copy(m_f, stf[0:1, :])
    rstdf = ln_rstd(stf[32:33, :], m_f, "l")
    mrf = pb.tile([1, SEG], bf16, tag="mr")
    nc.vector.tensor_tensor(mrf, m_f, rstdf, OP.mult)
    rbf = psB.tile([CH, SEG], f32, tag="mix", bufs=2, name="rbf")
    mm(rbf, onesr, rstdf, start=True, stop=True)
    cnf = pb.tile([CH, 2, SEG], bf16, tag="cn")
    for dd in range(2):
        nc.vector.tensor_tensor(cnf[:, dd, :], acc[:, dd, :], rbf, OP.mult)

    o_ps = psB.tile([CH, 2, SEG], f32, tag="r", bufs=1, name="o_ps")
    for mh in range(2):
        mm(o_ps[:, mh, :], wog(0, mh), cnf[:, 0, :], start=True, stop=False)
        mm(o_ps[:, mh, :], wog(1, mh), cnf[:, 1, :], start=False, stop=False)
        mm(o_ps[:, mh, :], wogsum(mh), mrf, start=False, stop=True)
    yt = pb.tile([CH, 2, SEG], f32, tag="yt")
    nc.vector.tensor_tensor(yt, o_ps, xfm_sb, OP.add)
    if not os.environ.get("DEBUG_RT"):
        dma(out=t["y"], in_=yt)

    for pool in (psB, pb, pa, own, consts):
        pool.release()
    lp.__exit__(None, None, None)


def _prep_inputs(inputs):
    """Host-side parameter folding + per-core input maps."""
    f = lambda a: np.asarray(a, dtype=np.float32)
    tobf = lambda a: np.ascontiguousarray(
        np.asarray(a, dtype=np.float32)).astype(ml_dtypes.bfloat16)
    x = f(inputs["x"])
    pe_w, pe_b = f(inputs["pe_w"]), f(inputs["pe_b"])
    tv_w, tv_b = f(inputs["tv_w"]), f(inputs["tv_b"])
    mq_w, mq_b = f(inputs["mq_w"]), f(inputs["mq_b"])
    ln_g, ln_b = f(inputs["ref_ln_g"]), f(inputs["ref_ln_b"])
    w1, b1 = f(inputs["ref_w1"]), f(inputs["ref_b1"])
    w2, b2 = f(inputs["ref_w2"]), f(inputs["ref_b2"])
    gw, gb = f(inputs["gate_w"]), f(inputs["gate_b"])
    og, ob = f(inputs["out_ln_g"]), f(inputs["out_ln_b"])
    ow, obias = f(inputs["out_w"]), f(inputs["out_b"])

    w1g = ln_g[:, :, None] * w1                       # (I, 264, 512)
    b1e = b1 + np.einsum("if,ifo->io", ln_b, w1)      # (I, 512)
    w2s = w2 / GSC
    wogm = og[:, None] * ow                           # (256, 256)
    boe = obias + ob @ ow

    # ---- blobA ----
    blobA = np.zeros((CH, A_COLS), np.float32)
    tvpe = np.concatenate([tv_w, pe_w], axis=1)       # (256, 288)
    blobA[:, A_TVPE:A_TVPE + 288] = tvpe[0:128]
    blobA[:, A_TVPE + 288:A_TVPE + 576] = tvpe[128:256]
    blobA[:, A_ID:A_ID + CH] = np.eye(CH)
    blobA[:, A_TRIL:A_TRIL + CH] = np.triu(np.ones((CH, CH)))
    blobA[:, A_OC264] = 1.0 / (D + H)
    blobA[:, A_OC256] = 1.0 / D
    blobA[:, A_ONEC] = 1.0
    blobA[0, A_ONER:A_ONER + CH] = 1.0
    blobA[0, A_PEBR:A_PEBR + P] = pe_b

    # ---- blobB ----
    blobB = np.zeros((CH, B_COLS), np.float32)
    for kh in range(2):
        blobB[:, B_PEW + kh * P:B_PEW + (kh + 1) * P] = \
            pe_w[kh * CH:(kh + 1) * CH]
        blobB[:, B_MQW + kh * H:B_MQW + (kh + 1) * H] = \
            mq_w[kh * CH:(kh + 1) * CH]
        for mh in range(2):
            blobB[:, B_WOG + (kh * 2 + mh) * CH:
                  B_WOG + (kh * 2 + mh + 1) * CH] = \
                wogm[kh * CH:(kh + 1) * CH, mh * CH:(mh + 1) * CH]
    for it in range(I):
        for kh in range(2):
            blobB[:, B_W1K + it * 1024 + kh * 512:
                  B_W1K + it * 1024 + (kh + 1) * 512] = \
                w1g[it, kh * CH:(kh + 1) * CH, :]
        for kh in range(4):
            for mh in range(2):
                blobB[:, B_W2K + it * 1024 + (kh * 2 + mh) * CH:
                      B_W2K + it * 1024 + (kh * 2 + mh + 1) * CH] = \
                    w2s[it, kh * CH:(kh + 1) * CH, mh * CH:(mh + 1) * CH]
        blobB[0:H, B_W1K2 + it * 512:B_W1K2 + (it + 1) * 512] = \
            w1g[it, D:D + H, :]
        blobB[0, B_W1SUM + it * 512:B_W1SUM + (it + 1) * 512] = \
            -w1g[it].sum(axis=0)
    for it in range(2):
        for kh in range(4):
            for mh in range(2):
                blobB[:, B_GWK + it * 1024 + (kh * 2 + mh) * CH:
                      B_GWK + it * 1024 + (kh * 2 + mh + 1) * CH] = \
                    gw[it, kh * CH:(kh + 1) * CH, mh * CH:(mh + 1) * CH]
    blobB[0, B_WOGSUM:B_WOGSUM + D] = -wogm.sum(axis=0)

    # ---- blobF (fp32) ----
    blobF = np.zeros((CH, F_COLS), np.float32)
    blobF[0:P, F_PEB] = pe_b
    blobF[0:H, F_MQB] = 0.5 * mq_b
    for it in range(I):
        blobF[:, F_B1 + it * 4:F_B1 + (it + 1) * 4] = \
            (GSC * b1e[it]).reshape(4, CH).T
        blobF[:, F_B2 + it * 2:F_B2 + (it + 1) * 2] = \
            b2[it].reshape(2, CH).T
    for it in range(2):
        blobF[:, F_GB + it * 2:F_GB + (it + 1) * 2] = \
            gb[it].reshape(2, CH).T

    blobA = tobf(blobA)
    blobB = tobf(blobB)
    tvb = np.ascontiguousarray(tv_b[None, :])

    in_maps = []
    for core in range(NCORES):
        b, pos = divmod(core, NCORES // B)
        s0 = pos * SEG
        xb = x[b]                                     # (L, D)
        # slot-permuted x, feature-major: prefix chunks then own chunks
        xp = np.zeros((L, D), np.float32)
        xp[0:s0] = xb[0:s0]
        xp[NPREF * CH:NSLOT * CH] = xb[s0:s0 + SEG]
        xp_fm = xp.T                                  # (D, 2048)
        blobC = np.zeros((CH, C_COLS), np.float32)
        blobC[:, C_X:C_X + 2048] = xp_fm[0:CH]
        blobC[:, C_X + 2048:C_X + 4096] = xp_fm[CH:2 * CH]
        gl = np.arange(s0, s0 + SEG, dtype=np.float64)
        iv = (1.0 / (np.sqrt(gl + 1.0) * math.sqrt(P))).astype(np.float32)
        blobC[0:2 * P, C_INVN:C_INVN + SEG] = iv[None, :]
        km = np.zeros(NSLOT, np.float32)
        km[0:4 * pos] = 1.0
        km[NPREF:] = 1.0
        blobC[:, C_KM:C_KM + NSLOT] = km[None, :]
        x_fm = np.zeros((CH, 2, SEG), np.float32)
        xo = xb[s0:s0 + SEG] + boe[None, :]           # (512, 256)
        x_fm[:, 0, :] = xo.T[0:CH]
        x_fm[:, 1, :] = xo.T[CH:2 * CH]
        m = {"blobA": blobA, "blobB": blobB, "blobF": blobF,
             "tvb": tvb, "x_fm": np.ascontiguousarray(x_fm),
             "blobC": tobf(blobC)}
        in_maps.append(m)
    return in_maps


def kernel(**inputs):
    from concourse.bass_utils import run_bass_kernel_spmd

    if "nc" not in _CACHE:
        _CACHE["nc"] = _build_program()
    nc = _CACHE["nc"]
    in_maps = _prep_inputs(inputs)
    res = run_bass_kernel_spmd(nc, in_maps, core_ids=list(range(NCORES)))
    out = np.empty((B, L, D), dtype=np.float32)
    for core in range(NCORES):
        b, pos = divmod(core, NCORES // B)
        s0 = pos * SEG
        y = np.asarray(res.results[core]["y"])        # (128, 2, 512)
        out[b, s0:s0 + SEG, :] = y.transpose(1, 0, 2).reshape(D, SEG).T
    return out


def gather(res):
    out = np.empty((B, L, D), dtype=np.float32)
    for core in range(NCORES):
        b, pos = divmod(core, NCORES // B)
        s0 = pos * SEG
        y = np.asarray(res.results[core]["y"])
        out[b, s0:s0 + SEG, :] = y.transpose(1, 0, 2).reshape(D, SEG).T
    return out


# revision 22
# speedup vs baseline: 1.3473x; 1.1016x over previous
"""Trainium2 Bass kernel for nn_AttentionGuidedIterativeBlock.

Math reformulation: the (B,L,P,D) phasor cumsum + retrieval is causal linear
attention with feature map Kf = [cos(phases), sin(phases)] (2P=64 dims):

    retrieved[l] = (sum_{l'<=l} (Qf[l].Kf[l']) * V[l']) / (sqrt(l+1)*sqrt(P))

Sharding: 8 cores x 512 tokens (cores 0-3 batch 0, 4-7 batch 1).  Each core
rebuilds the prefix state S = Kf^T @ V_aug over its batch prefix from a
per-core slot-permuted copy of x (12 prefix slots, zero-padded + masked via a
per-slot km scalar, then the 4 own chunks), so the program is uniform across
cores while own-segment K/V prep and prefix-state accumulation share the same
matmuls.  Everything runs in bf16 on the PE (fp32 PSUM accumulate); LayerNorm
mean-subtraction is folded into w1/wog as a rank-1 matmul (-mr x colsum(W)),
gelu(exact erf) is replaced by the sigmoid approximation expressed as
silu(1.702x)/1.702 so every activation function used stays inside one
activation-table set except nothing -- zero table swaps except none.
"""

import math
import os

import numpy as np
import ml_dtypes

D, P, I, H = 256, 32, 3, 8
B, L = 2, 2048
NCORES = 8
SEG = 512          # tokens per core
CH = 128           # chunk (tile partition) size
NSLOT = 16         # 12 prefix slots + 4 own chunks
NPREF = 12
HSEG = 256
PI = math.pi
EPS = 1e-5
GSC = 1.702        # sigmoid-gelu: gelu(x) ~= silu(GSC*x)/GSC

_CACHE = {}

# Source-content salt folded into a tensor name: any kernel.py change makes a
# distinct NEFF signature, so the neuron compile cache can never serve a NEFF
# built from a different version of this file.
import hashlib
with open(__file__, "rb") as _f:
    _SALT = hashlib.md5(_f.read()).hexdigest()[:8]

# ---- shared-blob column offsets (bf16, 128 partitions) ----
# blobA (early consts): tvpe (2,288), ident(128), tril(128), oc264, oc256,
#   onescol, onesrow(row0,128), pe_b_row(row0,32)
A_TVPE = 0
A_ID = 576
A_TRIL = 704
A_OC264 = 832
A_OC256 = 833
A_ONEC = 834
A_ONER = 835          # row0 cols 835:1347 = 1.0
A_PEBR = 1347         # row0 cols 1347:1379
A_MQBH = 1379         # row0 cols 1379:1387 = 0.5*mq_b
A_COLS = 1387

# blobB (weights): pe_w (2,32), mq_w (2,8), wog (2,2,128), w1k I*(2,512),
#   w2k I*(4,2,128), gwk 2*(4,2,128), w1k2 I*(8rows,512), w1sum_neg I*(row0,512),
#   wogsum_neg (row0, 256)
B_PEW = 0
B_MQW = 64
B_WOG = 80
B_W1K = 592
B_W2K = B_W1K + I * 1024          # 3664
B_GWK = B_W2K + I * 1024          # 6736
B_W1K2 = B_GWK + 2 * 1024         # 8784 (rows 0:8)
B_W1SUM = B_W1K2 + I * 512        # 10320 (row0)
B_WOGSUM = B_W1SUM + I * 512      # 11856 (row0)
B_COLS = B_WOGSUM + 256           # 12112

# blobF (fp32 biases): pe_b_col(32,1), mq_bh_col(8,1), b1s I*(128,4),
#   b2 I*(128,2), gb 2*(128,2)
F_PEB = 0
F_MQB = 1
F_B1 = 2
F_B2 = F_B1 + I * 4       # 14
F_GB = F_B2 + I * 2       # 20
F_COLS = F_GB + 4         # 24

# blobC (per-core): x_perm (2,2048), invn (rows0:64, 512), km (1? full col, 16)
C_X = 0
C_INVN = 4096
C_KM = 4608
C_COLS = 4624


def _patch_walrus_passes():
    import concourse.bass_utils as bu
    if getattr(bu, "_nv_patched", False):
        return
    orig = bu.run_command

    def patched(cmd, cwd=None, **kw):
        cmd = list(cmd)
        if "--pass" in cmd:
            i = cmd.index("--pass")
            cmd[i + 1] = cmd[i + 1].replace("birverifier,", "")
        return orig(cmd, cwd=cwd, **kw)

    bu.run_command = patched
    bu._nv_patched = True


def _build_program(split=True):
    _patch_walrus_passes()
    import concourse.bass as bass
    import concourse.tile as tile
    from concourse import mybir

    f32 = mybir.dt.float32

    nc = bass.Bass("TRN2", target_bir_lowering=False, debug=False,
                   num_devices=NCORES)

    def din(name, shape, dt):
        return nc.dram_tensor(name, shape, dt, kind="ExternalInput").ap()

    bf16 = mybir.dt.bfloat16
    t = {}
    t["blobA_" + _SALT] = din("blobA_" + _SALT, (CH, A_COLS), bf16)
    t["blobA"] = t["blobA_" + _SALT]
    t["blobB"] = din("blobB", (CH, B_COLS), bf16)
    t["blobC"] = din("blobC", (CH, C_COLS), bf16)
    t["blobF"] = din("blobF", (CH, F_COLS), f32)
    t["tvb"] = din("tvb", (1, D), f32)
    t["x_fm"] = din("x_fm", (CH, 2, SEG), f32)
    t["y"] = nc.dram_tensor("y", (CH, 2, SEG), f32, kind="ExternalOutput").ap()
    if os.environ.get("DEBUG_DUMP"):
        def dout(name, shape):
            t[name] = nc.dram_tensor(name, shape, f32,
                                     kind="ExternalOutput").ap()
        dout("d_qfs", (2 * P, SEG))
        dout("d_at", (H, SEG))
        dout("d_rt", (CH, 2, SEG))
        dout("d_cn", (CH, 2, SEG))
        dout("d_rstd", (2, SEG))
        dout("d_hh", (CH, 4, SEG))
        dout("d_rf", (CH, 2, SEG))
        dout("d_q1", (CH, 2, SEG))
        dout("d_sh", (2 * P, 2, D))

    with tile.TileContext(nc) as tc:
        _body(tc, nc, t, f32, bf16, bass, mybir)
    if split:
        _split_waits(nc, mybir)
    return nc


def _split_waits(nc, mybir, cap=1):
    """Move excess sync-waits onto preceding same-engine NOPs (this walrus
    build allows one wait slot per instruction)."""
    for fn in nc.m.functions:
        for blk in fn.blocks:
            out = []
            for ins in blk.instructions:
                si = ins.sync_info
                if si is not None and len(si.on_wait) > cap:
                    waits = list(si.on_wait)
                    extra, keep = waits[:-cap], waits[-cap:]
                    for j, w in enumerate(extra):
                        nop = mybir.InstNoOp(name=f"{ins.name}_wsplit{j}",
                                             ins=[], outs=[])
                        nop.engine = ins.engine
                        nop.sync_info = mybir.SyncInfo(on_wait=[w],
                                                       on_update=[])
                        out.append(nop)
                    ins.sync_info = mybir.SyncInfo(on_wait=keep,
                                                   on_update=si.on_update)
                out.append(ins)
            blk.instructions = out


def _body(tc, nc, t, f32, bf16, bass, mybir):
    from concourse.alu_op_type import AluOpType as OP

    AF = mybir.ActivationFunctionType
    AX = mybir.AxisListType.X

    lp = nc.allow_low_precision(reason="bf16 kernel; tolerance 2e-2 validated")
    lp.__enter__()
    consts = tc.alloc_tile_pool(name="consts", bufs=1)
    own = tc.alloc_tile_pool(name="own", bufs=1)
    pa = tc.alloc_tile_pool(name="pa", bufs=2)
    pb = tc.alloc_tile_pool(name="pb", bufs=2)

    dma = nc.sync.dma_start
    mm = nc.tensor.matmul

    # ---- constant blobs (6 input DMAs total) ----
    cA = consts.tile([CH, A_COLS], bf16)
    dma(out=cA, in_=t["blobA"])
    cC = consts.tile([CH, C_COLS], bf16)
    dma(out=cC, in_=t["blobC"])
    cB = consts.tile([CH, B_COLS], bf16)
    dma(out=cB, in_=t["blobB"])
    cF = consts.tile([CH, F_COLS], f32)
    dma(out=cF, in_=t["blobF"])
    tvb_bc = consts.tile([2 * P, D], f32)
    dma(out=tvb_bc, in_=t["tvb"].to_broadcast((2 * P, D)))
    xfm_sb = consts.tile([CH, 2, SEG], f32)
    dma(out=xfm_sb, in_=t["x_fm"])

    def xslot(kh, s):
        return cC[:, C_X + kh * 2048 + s * CH: C_X + kh * 2048 + (s + 1) * CH]

    tvpe = lambda kh: cA[:, A_TVPE + kh * 288: A_TVPE + (kh + 1) * 288]
    ident = cA[:, A_ID:A_ID + CH]
    tril = cA[:, A_TRIL:A_TRIL + CH]
    oc264 = cA[:, A_OC264:A_OC264 + 1]
    oc256 = cA[:, A_OC256:A_OC256 + 1]
    onesr = cA[0:1, A_ONER:A_ONER + CH]
    onesr512 = cA[0:1, A_ONER:A_ONER + SEG]
    mqbh_row = cA[0:1, A_MQBH:A_MQBH + H]
    pebr = cA[0:1, A_PEBR:A_PEBR + P]
    invn = cC[0:2 * P, C_INVN:C_INVN + SEG]
    pe_b_col = cF[0:P, F_PEB:F_PEB + 1]
    mq_bh_col = cF[0:H, F_MQB:F_MQB + 1]
    halfpi = consts.tile([CH, 1], f32)
    nc.vector.memset(halfpi, PI / 2)
    i32 = mybir.dt.int32
    qmagic = consts.tile([1, SEG], i32)
    nc.vector.memset(qmagic, 0x5F3759DF)
    ones8f = consts.tile([H, 1], f32)
    nc.vector.memset(ones8f, 1.0)
    ones1x8f = consts.tile([1, H], f32)
    nc.vector.memset(ones1x8f, 1.0)
    eps_row = consts.tile([1, SEG], bf16)
    nc.vector.memset(eps_row, EPS)
    c264 = consts.tile([1, 1], f32)
    nc.vector.memset(c264, 1.0 / (D + H))
    f32r = mybir.dt.float32r
    rr = lambda ap: ap.bitcast(f32r)

    def ln_rstd(st2_psum, msq, tagp, ln=SEG):
        # rstd = 1/sqrt(st2 - m^2), DVE-only (Quake seed + 1 Newton);
        # eps is pre-accumulated into st2 via a K=1 matmul.
        var = pb.tile([1, ln], f32, tag=tagp + "var")
        nc.vector.tensor_tensor(var, st2_psum, msq, OP.subtract)
        sh = pb.tile([1, ln], i32, tag=tagp + "sh")
        nc.vector.tensor_scalar(sh, var.bitcast(i32), 1, None,
                                OP.logical_shift_right)
        si = pb.tile([1, ln], i32, tag=tagp + "si")
        nc.vector.tensor_tensor(si, qmagic[:, 0:ln], sh, OP.subtract)
        y = si.bitcast(f32)
        y2 = pb.tile([1, ln], f32, tag=tagp + "y2")
        nc.scalar.activation(y2, y, AF.Square)
        tn = pb.tile([1, ln], f32, tag=tagp + "tn")
        nc.vector.tensor_tensor(tn, y2, var, OP.mult)
        un = pb.tile([1, ln], f32, tag=tagp + "un")
        nc.vector.tensor_scalar(un, tn, -0.5, 1.5, OP.mult, OP.add)
        rstd = pb.tile([1, ln], bf16, tag=tagp + "rstd")
        nc.vector.tensor_tensor(rstd, y, un, OP.mult)
        return rstd

    pe_w = lambda kh: cB[:, B_PEW + kh * P: B_PEW + (kh + 1) * P]
    mq_w = lambda kh: cB[:, B_MQW + kh * H: B_MQW + (kh + 1) * H]
    wog = lambda kh, mh: cB[:, B_WOG + (kh * 2 + mh) * CH:
                            B_WOG + (kh * 2 + mh + 1) * CH]
    w1k = lambda it, kh, o: cB[:, B_W1K + it * 1024 + kh * 512 + o * CH:
                               B_W1K + it * 1024 + kh * 512 + (o + 1) * CH]
    w2k = lambda it, kh, mh: cB[:, B_W2K + it * 1024 + (kh * 2 + mh) * CH:
                                B_W2K + it * 1024 + (kh * 2 + mh + 1) * CH]
    gwk = lambda it, kh, mh: cB[:, B_GWK + it * 1024 + (kh * 2 + mh) * CH:
                                B_GWK + it * 1024 + (kh * 2 + mh + 1) * CH]
    w1k29 = lambda it, o: cB[0:33, B_W1K2 + it * 512 + o * CH:
                             B_W1K2 + it * 512 + (o + 1) * CH]
    wogsum = lambda mh: cB[0:1, B_WOGSUM + mh * CH:B_WOGSUM + (mh + 1) * CH]

    # =========== phase A: prefix state + own K/V (unified slot loop) =======
    psA = tc.alloc_tile_pool(name="psA", bufs=1, space="PSUM")

    S_ps = psA.tile([2 * P, D + 8], f32, tag="S")
    own_kf = own.tile([CH, 4, 2 * P], bf16)    # own kf, token-major, unmasked
    v_own = own.tile([CH, 4, D + 8], bf16)     # own V_aug, token-major

    for g in range(4):
        vq_ps = psA.tile([CH, 4, 512], f32, tag="vq", bufs=1,
                         name=f"vq{g}")
        for c in range(4):
            s = 4 * g + c
            mm(vq_ps[:, c, 0:D + P], xslot(0, s), tvpe(0),
               start=True, stop=False, skip_group_check=True)
            mm(vq_ps[:, c, 0:D + P], xslot(1, s), tvpe(1),
               start=False, stop=False, skip_group_check=True)
            mm(vq_ps[:, c, D:D + P], onesr, pebr, start=False, stop=True,
               skip_group_check=True)
        tq = pa.tile([CH, 4, P], bf16, tag="tq")
        nc.scalar.activation(tq, vq_ps[:, :, D:D + P], AF.Tanh)
        aq = pa.tile([CH, 4, P], bf16, tag="aq")
        nc.scalar.activation(aq, tq, AF.Abs)
        kfg = own_kf if g == 3 else pa.tile([CH, 4, 2 * P], bf16, tag="kf")
        nc.scalar.activation(kfg[:, :, 0:P], aq, AF.Sin, scale=-PI,
                             bias=halfpi)
        nc.scalar.activation(kfg[:, :, P:2 * P], tq, AF.Sin, scale=PI)
        vdst = v_own if g == 3 else pa.tile([CH, 4, D + 8], bf16, tag="vsb")
        nc.vector.tensor_copy(vdst[:, :, 0:D], vq_ps[:, :, 0:D])
        nc.vector.memset(vdst[:, :, D:D + 8], 1.0)
        if g < 3:
            kfm = pa.tile([CH, 4, 2 * P], bf16, tag="kfm")
            nc.vector.tensor_tensor(
                kfm, kfg,
                cC[:, C_KM + 4 * g:C_KM + 4 * g + 4].unsqueeze(-1)
                .broadcast_to([CH, 4, 2 * P]), OP.mult)
            for c in range(4):
                s = 4 * g + c
                mm(S_ps, kfm[:, c, :], vdst[:, c, :],
                   start=(s == 0), stop=(s == NPREF - 1),
                   skip_group_check=True)

    # S_h0 = prefix state (+ rank-1 tv_b fold);  S_h1 adds own chunks 0,1
    S_h = []
    for hi in range(2):
        if hi == 1:
            for c in range(2):
                mm(S_ps, own_kf[:, c, :], v_own[:, c, :],
                   start=False, stop=(c == 1), skip_group_check=True)
        kfsum = own.tile([2 * P, 1], f32, tag=f"kfsum{hi}")
        nc.vector.tensor_copy(kfsum, S_ps[:, D:D + 1])
        st = own.tile([2 * P, D], f32, tag=f"S_tmp{hi}")
        nc.vector.tensor_tensor(
            st, tvb_bc, kfsum.broadcast_to([2 * P, D]), OP.mult)
        sh = own.tile([2 * P, D], bf16, tag=f"S_h{hi}")
        nc.vector.tensor_tensor(sh, st, S_ps[:, 0:D], OP.add)
        S_h.append(sh)

    # kff: own kf feature-major (for intra scores + iteration-0 Qf)
    kff = own.tile([2 * P, SEG], bf16)
    for c in range(4):
        tr_ps = psA.tile([2 * P, CH], bf16, tag="tr", bufs=2, name="tr_ps")
        nc.tensor.transpose(tr_ps, own_kf[:, c, :], ident)
        nc.vector.tensor_copy(kff[:, c * CH:(c + 1) * CH], tr_ps)

    acc = own.tile([CH, 2, SEG], bf16)
    nc.vector.memset(acc, 0.0)
    qA = own.tile([CH, 2, SEG], bf16)
    qB = own.tile([CH, 2, SEG], bf16)
    # initial query = own x (bf16, feature-major) = slots 12-15 of x_perm
    for kh in range(2):
        nc.vector.tensor_copy(
            qA[:, kh, :],
            cC[:, C_X + kh * 2048 + NPREF * CH:C_X + kh * 2048 + NSLOT * CH])

    psA.release()
    psB = tc.alloc_tile_pool(name="psB", bufs=1, space="PSUM")

    # intra score blocks per half: (key chunk, local query lo, n)
    HALF_BLOCKS = {0: [(0, 0, 2 * CH), (1, CH, CH)],
                   1: [(2, 0, 2 * CH), (3, CH, CH)]}

    # =========== refinement iterations (full-width, feature-major) ========
    for it in range(I):
        q = qA if it % 2 == 0 else qB
        qn = qB if it % 2 == 0 else qA

        # Qf (feature-major), with 1/norm folded
        if it > 0:
            qp_ps = psB.tile([P, SEG], f32, tag="mix", bufs=2, name="qp_ps")
            mm(qp_ps, pe_w(0), q[:, 0, :], start=True, stop=False)
            mm(qp_ps, pe_w(1), q[:, 1, :], start=False, stop=True)
            tq_ = pb.tile([P, SEG], bf16, tag="tq")
            nc.scalar.activation(tq_, qp_ps, AF.Tanh, bias=pe_b_col)
            aq_ = pb.tile([P, SEG], bf16, tag="aq")
            nc.scalar.activation(aq_, tq_, AF.Abs)
            qf = pb.tile([2 * P, SEG], bf16, tag="qf")
            nc.scalar.activation(qf[0:P, :], aq_, AF.Sin, scale=-PI,
                                 bias=halfpi[0:P, :])
            nc.scalar.activation(qf[P:2 * P, :], tq_, AF.Sin, scale=PI)
        else:
            qf = kff
        qfs = pb.tile([2 * P, SEG], bf16, tag="qfs")
        nc.vector.tensor_tensor(qfs, qf, invn, OP.mult)
        if os.environ.get("DEBUG_DUMP") and it == 0:
            dq = consts.tile([2 * P, SEG], f32, tag="dbg_q")
            nc.vector.tensor_copy(dq, qfs)
            dma(out=t["d_qfs"], in_=dq)
            ds0 = consts.tile([2 * P, 2, D], f32, tag="dbg_s")
            nc.vector.tensor_copy(ds0[:, 0, :], S_h[0])
            nc.vector.tensor_copy(ds0[:, 1, :], S_h[1])
            dma(out=t["d_sh"], in_=ds0)

        # attention: feature-major logits, token-major tanh-softmax
        # (keeps reciprocals at 8/32 elems per DVE lane), mq_b/2 folded in
        # via a K=1 rank-1 matmul
        z_ps = psB.tile([H, SEG], f32, tag="mix", bufs=2, name="z_ps")
        mm(z_ps, mq_w(0), q[:, 0, :], start=True, stop=False,
           skip_group_check=True)
        mm(z_ps, mq_w(1), q[:, 1, :], start=False, stop=False,
           skip_group_check=True)
        mm(z_ps, mqbh_row, onesr512, start=False, stop=True,
           skip_group_check=True)
        z_sb = pb.tile([H, SEG], bf16, tag="z_sb")
        nc.vector.tensor_scalar(z_sb, z_ps, 0.5, None, OP.mult)
        ztm_ps = psB.tile([CH, 4, H], bf16, tag="sc", bufs=1, name="ztm")
        for c in range(4):
            nc.tensor.transpose(ztm_ps[:, c, :],
                                z_sb[:, c * CH:(c + 1) * CH],
                                ident[0:H, 0:H])
        tht = pb.tile([CH, 4, H], f32, tag="tht")
        nc.scalar.activation(tht, ztm_ps, AF.Tanh)
        num = pb.tile([CH, 4, H], f32, tag="num")
        nc.vector.tensor_scalar_add(num, tht, 1.0)
        den = pb.tile([CH, 4, H], f32, tag="den")
        nc.vector.tensor_scalar(den, tht, -1.0, 1.0, OP.mult, OP.add)
        rec = pb.tile([CH, 4, H], f32, tag="rec")
        nc.vector.reciprocal(rec, den)
        ex = pb.tile([CH, 4, H], f32, tag="ex")
        nc.vector.tensor_tensor(ex, num, rec, OP.mult)
        es = pb.tile([CH, 4], f32, tag="es")
        nc.vector.tensor_reduce(es, ex, AX, OP.add)
        esr = pb.tile([CH, 4], f32, tag="esr")
        nc.vector.reciprocal(esr, es)
        at_tm = pb.tile([CH, 4, H], bf16, tag="at_tm")
        nc.vector.tensor_tensor(at_tm, ex,
                                esr.unsqueeze(-1).broadcast_to([CH, 4, H]),
                                OP.mult)
        atf_ps = psB.tile([H, SEG], bf16, tag="sc", bufs=1, name="atf")
        for c in range(4):
            nc.tensor.transpose(atf_ps[:, c * CH:(c + 1) * CH],
                                at_tm[:, c, :], ident)
        at = pb.tile([H, SEG], bf16, tag="at")
        nc.vector.tensor_copy(at, atf_ps)
        if os.environ.get("DEBUG_DUMP") and it == 0:
            da = consts.tile([H, SEG], f32, tag="dbg_a")
            nc.vector.tensor_copy(da, at)
            dma(out=t["d_at"], in_=da)
        sqa = pb.tile([H, SEG], bf16, tag="sqa")
        nc.scalar.activation(sqa, at, AF.Square)

        # two half-segment pipelines: retrieval -> LN -> MLP -> gate,
        # h1's PE work overlaps h0's DVE/ACT chains
        rf = pb.tile([CH, 2, SEG], bf16, tag="rf")
        gd = pb.tile([CH, 2, SEG], bf16, tag="gd")
        for h in range(2):
            hsl = slice(h * HSEG, (h + 1) * HSEG)
            r_ps = psB.tile([CH, 2, HSEG], f32, tag="r", bufs=1,
                            name=f"r_ps{h}")
            for dd in range(2):
                # single accumulation group per PSUM bank: has_written is
                # per (partition, bank); only the first matmul may start
                mm(r_ps[:, dd, :], S_h[h][:, dd * CH:(dd + 1) * CH],
                   qfs[:, hsl], start=(dd == 0), stop=False,
                   skip_group_check=True)
            for bi, (kc, lo, n) in enumerate(HALF_BLOCKS[h]):
                qsl = slice(lo, lo + n)
                gqsl = slice(h * HSEG + lo, h * HSEG + lo + n)
                sc_ps = psB.tile([CH, 2 * CH], f32, tag="sc", bufs=1,
                                 name="sc_ps")
                mm(sc_ps[:, 0:n], kff[:, kc * CH:(kc + 1) * CH],
                   qfs[:, gqsl], start=True, stop=True)
                sc_sb = pb.tile([CH, 2 * CH], bf16, tag="sc_sb")
                nc.vector.tensor_tensor(sc_sb[:, 0:CH], sc_ps[:, 0:CH],
                                        tril, OP.mult)
                if n > CH:
                    nc.vector.tensor_copy(sc_sb[:, CH:n], sc_ps[:, CH:n])
                last = bi == len(HALF_BLOCKS[h]) - 1
                for dd in range(2):
                    mm(r_ps[:, dd, qsl], v_own[:, kc, dd * CH:(dd + 1) * CH],
                       sc_sb[:, 0:n], start=False,
                       stop=(last and dd == 1), skip_group_check=True)

            rt = pb.tile([CH, 2, HSEG], bf16, tag="rt")
            nc.vector.tensor_copy(rt, r_ps)
            if os.environ.get("DEBUG_DUMP") and it == 0:
                drt = consts.tile([CH, 2, HSEG], f32, tag=f"dbg_rt{h}")
                nc.vector.tensor_copy(drt, rt)
                dma(out=t["d_rt"][:, :, h * HSEG:(h + 1) * HSEG], in_=drt)
            sq = pb.tile([CH, 2, HSEG], bf16, tag="sq")
            nc.scalar.activation(sq, r_ps, AF.Square)

            st_ps = psB.tile([33, HSEG], f32, tag="st", bufs=2,
                             name=f"st_ps{h}")
            mm(st_ps[0:1, :], oc264, rt[:, 0, :], start=True, stop=False,
               skip_group_check=True)
            mm(st_ps[0:1, :], oc264, rt[:, 1, :], start=False, stop=True,
               skip_group_check=True)
            mm(st_ps[32:33, :], oc264, sq[:, 0, :], start=True, stop=False,
               skip_group_check=True)
            mm(st_ps[32:33, :], oc264, sq[:, 1, :], start=False, stop=False,
               skip_group_check=True)
            mm(st_ps[32:33, :], oc264[0:H, :], sqa[:, hsl], start=False,
               stop=False, skip_group_check=True)
            mm(st_ps[32:33, :], onesr[0:1, 0:1], eps_row[:, 0:HSEG],
               start=False, stop=True, skip_group_check=True)
            msq = pb.tile([1, HSEG], f32, tag="lmsq")
            nc.scalar.activation(msq, st_ps[0:1, :], AF.Square, bias=c264)
            m_b = pb.tile([1, HSEG], bf16, tag="m_b")
            nc.vector.tensor_scalar_add(m_b, st_ps[0:1, :], 1.0 / (D + H))
            rstd = ln_rstd(st_ps[32:33, :], msq, "l", HSEG)
            rbb = psB.tile([CH, HSEG], f32, tag="st", bufs=2,
                           name=f"rbb{h}")
            mm(rbb, onesr, rstd, start=True, stop=True)
            rb_sb = pb.tile([CH, HSEG], bf16, tag="rb_sb")
            nc.vector.tensor_copy(rb_sb, rbb)

            cn = pb.tile([CH, 2, HSEG], bf16, tag="cn")
            for dd in range(2):
                nc.vector.tensor_tensor(cn[:, dd, :], rt[:, dd, :], rb_sb,
                                        OP.mult)
            if os.environ.get("DEBUG_DUMP") and it == 0:
                dcn = consts.tile([CH, 2, HSEG], f32, tag=f"dbg_cn{h}")
                nc.vector.tensor_copy(dcn, cn)
                dma(out=t["d_cn"][:, :, h * HSEG:(h + 1) * HSEG], in_=dcn)
                drs = consts.tile([1, HSEG], f32, tag=f"dbg_rs{h}")
                nc.vector.tensor_copy(drs, rstd)
                dma(out=t["d_rstd"][h:h + 1, 0:HSEG], in_=drs)
            cnam = pb.tile([33, HSEG], bf16, tag="cna")
            nc.vector.memset(cnam, 0.0)
            nc.vector.tensor_tensor(cnam[0:H, :], at[:, hsl], rb_sb[0:H, :],
                                    OP.mult)
            nc.vector.tensor_tensor(cnam[32:33, :], m_b, rstd, OP.mult)

            hh = pb.tile([CH, 4, HSEG], bf16, tag="h")
            for o in range(4):
                h_ps = psB.tile([CH, HSEG], f32, tag="hps", bufs=2,
                                name="h_ps")
                mm(h_ps, w1k(it, 0, o), cn[:, 0, :], start=True, stop=False)
                mm(h_ps, w1k29(it, o), cnam, start=False, stop=False)
                mm(h_ps, w1k(it, 1, o), cn[:, 1, :], start=False, stop=True)
                nc.scalar.activation(hh[:, o, :], h_ps, AF.Silu, scale=GSC,
                                     bias=cF[:, F_B1 + it * 4 + o:
                                             F_B1 + it * 4 + o + 1])
                if os.environ.get("DEBUG_DUMP") and it == 0:
                    dh = consts.tile([CH, HSEG], f32, tag=f"dbg_h{h}{o}")
                    nc.vector.tensor_copy(dh, hh[:, o, :])
                    dma(out=t["d_hh"][:, o, h * HSEG:(h + 1) * HSEG], in_=dh)

            for mh in range(2):
                rf_ps = psB.tile([CH, HSEG], f32, tag="hps", bufs=2,
                                 name="rf_ps")
                for kh in range(4):
                    mm(rf_ps, w2k(it, kh, mh), hh[:, kh, :],
                       start=(kh == 0), stop=(kh == 3))
                nc.scalar.activation(rf[:, mh, hsl], rf_ps, AF.Identity,
                                     bias=cF[:, F_B2 + it * 2 + mh:
                                             F_B2 + it * 2 + mh + 1])
            nc.vector.tensor_tensor(acc[:, :, hsl], acc[:, :, hsl],
                                    rf[:, :, hsl], OP.add)

            if it < I - 1:
                for mh in range(2):
                    g_ps = psB.tile([CH, HSEG], f32, tag="hps", bufs=2,
                                    name="g_ps")
                    for kh in range(4):
                        rhs = (q[:, kh, hsl] if kh < 2
                               else rf[:, kh - 2, hsl])
                        mm(g_ps, gwk(it, kh, mh), rhs,
                           start=(kh == 0), stop=(kh == 3))
                    nc.scalar.activation(gd[:, mh, hsl], g_ps, AF.Tanh,
                                         bias=cF[:, F_GB + it * 2 + mh:
                                                 F_GB + it * 2 + mh + 1])
        if it < I - 1:
            nc.vector.tensor_tensor(qn, q, gd, OP.add)
        if os.environ.get("DEBUG_DUMP") and it == 0:
            drf = consts.tile([CH, 2, SEG], f32, tag="dbg_rf")
            nc.vector.tensor_copy(drf, rf)
            dma(out=t["d_rf"], in_=drf)
            dq1 = consts.tile([CH, 2, SEG], f32, tag="dbg_q1")
            nc.vector.tensor_copy(dq1, qn)
            dma(out=t["d_q1"], in_=dq1)

    # =========== final LN(acc) @ wog + x (+boe) ===========
    sqf = pb.tile([CH, 2, SEG], bf16, tag="sq")
    nc.scalar.activation(sqf, acc, AF.Square)
    stf = psB.tile([33, SEG], f32, tag="st", bufs=2, name="stf")
    mm(stf[0:1, :], oc256, acc[:, 0, :], start=True, stop=False,
       skip_group_check=True)
    mm(stf[0:1, :], oc256, acc[:, 1, :], start=False, stop=True,
       skip_group_check=True)
    mm(stf[32:33, :], oc256, sqf[:, 0, :], start=True, stop=False,
       skip_group_check=True)
    mm(stf[32:33, :], oc256, sqf[:, 1, :], start=False, stop=False,
       skip_group_check=True)
    mm(stf[32:33, :], onesr[0:1, 0:1], eps_row, start=False, stop=True,
       skip_group_check=True)
    msqf = pb.tile([1, SEG], f32, tag="lmsq")
    nc.scalar.activation(msqf, stf[0:1, :], AF.Square)
    m_f = pb.tile([1, SEG], bf16, tag="m_b")
    nc.vector.tensor_copy(m_f, stf[0:1, :])
    rstdf = ln_rstd(stf[32:33, :], msqf, "l")
    mrf = pb.tile([1, SEG], bf16, tag="mr")
    nc.vector.tensor_tensor(mrf, m_f, rstdf, OP.mult)
    rbf = psB.tile([CH, SEG], f32, tag="st", bufs=2, name="rbf")
    mm(rbf, onesr, rstdf, start=True, stop=True)
    rbf_sb = pb.tile([CH, SEG], bf16, tag="rb_sb")
    nc.vector.tensor_copy(rbf_sb, rbf)
    cnf = pb.tile([CH, 2, SEG], bf16, tag="cn")
    for dd in range(2):
        nc.vector.tensor_tensor(cnf[:, dd, :], acc[:, dd, :], rbf_sb,
                                OP.mult)

    yt = pb.tile([CH, 2, SEG], f32, tag="yt")
    for mh in range(2):
        o_ps = psB.tile([CH, SEG], f32, tag="hps", bufs=2, name="o_ps")
        mm(o_ps, wog(0, mh), cnf[:, 0, :], start=True, stop=False)
        mm(o_ps, wog(1, mh), cnf[:, 1, :], start=False, stop=False)
        mm(o_ps, wogsum(mh), mrf, start=False, stop=True)
        nc.vector.tensor_tensor(yt[:, mh, :], o_ps, xfm_sb[:, mh, :], OP.add)
    if not os.environ.get("DEBUG_RT"):
        dma(out=t["y"], in_=yt)

    for pool in (psB, pb, pa, own, consts):
        pool.release()
    lp.__exit__(None, None, None)


def _prep_inputs(inputs):
    """Host-side parameter folding + per-core input maps."""
    f = lambda a: np.asarray(a, dtype=np.float32)
    tobf = lambda a: np.ascontiguousarray(
        np.asarray(a, dtype=np.float32)).astype(ml_dtypes.bfloat16)
    x = f(inputs["x"])
    pe_w, pe_b = f(inputs["pe_w"]), f(inputs["pe_b"])
    tv_w, tv_b = f(inputs["tv_w"]), f(inputs["tv_b"])
    mq_w, mq_b = f(inputs["mq_w"]), f(inputs["mq_b"])
    ln_g, ln_b = f(inputs["ref_ln_g"]), f(inputs["ref_ln_b"])
    w1, b1 = f(inputs["ref_w1"]), f(inputs["ref_b1"])
    w2, b2 = f(inputs["ref_w2"]), f(inputs["ref_b2"])
    gw, gb = f(inputs["gate_w"]), f(inputs["gate_b"])
    og, ob = f(inputs["out_ln_g"]), f(inputs["out_ln_b"])
    ow, obias = f(inputs["out_w"]), f(inputs["out_b"])

    w1g = ln_g[:, :, None] * w1                       # (I, 264, 512)
    b1e = b1 + np.einsum("if,ifo->io", ln_b, w1)      # (I, 512)
    w2s = w2 / GSC
    wogm = og[:, None] * ow                           # (256, 256)
    boe = obias + ob @ ow

    # ---- blobA ----
    blobA = np.zeros((CH, A_COLS), np.float32)
    tvpe = np.concatenate([tv_w, pe_w], axis=1)       # (256, 288)
    blobA[:, A_TVPE:A_TVPE + 288] = tvpe[0:128]
    blobA[:, A_TVPE + 288:A_TVPE + 576] = tvpe[128:256]
    blobA[:, A_ID:A_ID + CH] = np.eye(CH)
    blobA[:, A_TRIL:A_TRIL + CH] = np.triu(np.ones((CH, CH)))
    blobA[:, A_OC264] = 1.0 / (D + H)
    blobA[:, A_OC256] = 1.0 / D
    blobA[:, A_ONEC] = 1.0
    blobA[0, A_ONER:A_ONER + 4 * CH] = 1.0
    blobA[0, A_PEBR:A_PEBR + P] = pe_b
    blobA[0, A_MQBH:A_MQBH + H] = mq_b

    # ---- blobB ----
    blobB = np.zeros((CH, B_COLS), np.float32)
    for kh in range(2):
        blobB[:, B_PEW + kh * P:B_PEW + (kh + 1) * P] = \
            pe_w[kh * CH:(kh + 1) * CH]
        blobB[:, B_MQW + kh * H:B_MQW + (kh + 1) * H] = \
            mq_w[kh * CH:(kh + 1) * CH]
        for mh in range(2):
            blobB[:, B_WOG + (kh * 2 + mh) * CH:
                  B_WOG + (kh * 2 + mh + 1) * CH] = \
                wogm[kh * CH:(kh + 1) * CH, mh * CH:(mh + 1) * CH]
    for it in range(I):
        for kh in range(2):
            blobB[:, B_W1K + it * 1024 + kh * 512:
                  B_W1K + it * 1024 + (kh + 1) * 512] = \
                w1g[it, kh * CH:(kh + 1) * CH, :]
        for kh in range(4):
            for mh in range(2):
                blobB[:, B_W2K + it * 1024 + (kh * 2 + mh) * CH:
                      B_W2K + it * 1024 + (kh * 2 + mh + 1) * CH] = \
                    w2s[it, kh * CH:(kh + 1) * CH, mh * CH:(mh + 1) * CH]
        blobB[0:H, B_W1K2 + it * 512:B_W1K2 + (it + 1) * 512] = \
            w1g[it, D:D + H, :]
        blobB[32, B_W1K2 + it * 512:B_W1K2 + (it + 1) * 512] = \
            -w1g[it].sum(axis=0)
    for it in range(2):
        for kh in range(4):
            for mh in range(2):
                blobB[:, B_GWK + it * 1024 + (kh * 2 + mh) * CH:
                      B_GWK + it * 1024 + (kh * 2 + mh + 1) * CH] = \
                    gw[it, kh * CH:(kh + 1) * CH, mh * CH:(mh + 1) * CH]
    blobB[0, B_WOGSUM:B_WOGSUM + D] = -wogm.sum(axis=0)

    # ---- blobF (fp32) ----
    blobF = np.zeros((CH, F_COLS), np.float32)
    blobF[0:P, F_PEB] = pe_b
    blobF[0:H, F_MQB] = 0.5 * mq_b
    for it in range(I):
        blobF[:, F_B1 + it * 4:F_B1 + (it + 1) * 4] = \
            (GSC * b1e[it]).reshape(4, CH).T
        blobF[:, F_B2 + it * 2:F_B2 + (it + 1) * 2] = \
            b2[it].reshape(2, CH).T
    for it in range(2):
        blobF[:, F_GB + it * 2:F_GB + (it + 1) * 2] = \
            gb[it].reshape(2, CH).T

    blobA = tobf(blobA)
    blobB = tobf(blobB)
    tvb = np.ascontiguousarray(tv_b[None, :])

    in_maps = []
    for core in range(NCORES):
        b, pos = divmod(core, NCORES // B)
        s0 = pos * SEG
        xb = x[b]                                     # (L, D)
        # slot-permuted x, feature-major: prefix chunks then own chunks
        xp = np.zeros((L, D), np.float32)
        xp[0:s0] = xb[0:s0]
        xp[NPREF * CH:NSLOT * CH] = xb[s0:s0 + SEG]
        xp_fm = xp.T                                  # (D, 2048)
        blobC = np.zeros((CH, C_COLS), np.float32)
        blobC[:, C_X:C_X + 2048] = xp_fm[0:CH]
        blobC[:, C_X + 2048:C_X + 4096] = xp_fm[CH:2 * CH]
        gl = np.arange(s0, s0 + SEG, dtype=np.float64)
        iv = (1.0 / (np.sqrt(gl + 1.0) * math.sqrt(P))).astype(np.float32)
        blobC[0:2 * P, C_INVN:C_INVN + SEG] = iv[None, :]
        km = np.zeros(NSLOT, np.float32)
        km[0:4 * pos] = 1.0
        km[NPREF:] = 1.0
        blobC[:, C_KM:C_KM + NSLOT] = km[None, :]
        x_fm = np.zeros((CH, 2, SEG), np.float32)
        xo = xb[s0:s0 + SEG] + boe[None, :]           # (512, 256)
        x_fm[:, 0, :] = xo.T[0:CH]
        x_fm[:, 1, :] = xo.T[CH:2 * CH]
        m = {"blobA_" + _SALT: blobA, "blobB": blobB, "blobF": blobF,
             "tvb": tvb, "x_fm": np.ascontiguousarray(x_fm),
             "blobC": tobf(blobC)}
        in_maps.append(m)
    return in_maps


def kernel(**inputs):
    from concourse.bass_utils import run_bass_kernel_spmd

    if "nc" not in _CACHE:
        _CACHE["nc"] = _build_program()
    nc = _CACHE["nc"]
    in_maps = _prep_inputs(inputs)
    res = run_bass_kernel_spmd(nc, in_maps, core_ids=list(range(NCORES)))
    out = np.empty((B, L, D), dtype=np.float32)
    for core in range(NCORES):
        b, pos = divmod(core, NCORES // B)
        s0 = pos * SEG
        y = np.asarray(res.results[core]["y"])        # (128, 2, 512)
        out[b, s0:s0 + SEG, :] = y.transpose(1, 0, 2).reshape(D, SEG).T
    return out


def gather(res):
    out = np.empty((B, L, D), dtype=np.float32)
    for core in range(NCORES):
        b, pos = divmod(core, NCORES // B)
        s0 = pos * SEG
        y = np.asarray(res.results[core]["y"])
        out[b, s0:s0 + SEG, :] = y.transpose(1, 0, 2).reshape(D, SEG).T
    return out
